# revision 3
# baseline (speedup 1.0000x reference)
"""CriticSwapGNN Trainium2 fused kernel: feat + 4 GAT layers + MLP head + pool
in ONE launch across 8 cores, with on-device AllGather of xp between layers.

Sharding: dst-range ownership, 8 cores x 49 blocks of 128 nodes. Edges sorted
by dst block, split lo/hi by src half (int16 gather indices), tiled 128/tile,
grouped in chunks of CHUNK_BLKS dst blocks. Table and matmul operands in bf16
(rel err ~4e-3 vs 2e-2 gate); accumulations in fp32 PSUM.
"""
import sys
import time
import numpy as np
import ml_dtypes

if '/opt/trn_rl_repo' not in sys.path:
    sys.path.insert(0, '/opt/trn_rl_repo')

N = 50000; E = 800000; F = 16; HID = 128; H = 4; C = 32; FC = 256; NL = 15; NG = 8
NCORES = 8
BLK = 128
BPC = 49                      # blocks per core
NBLK = NCORES * BPC           # 392
NPAD = NBLK * BLK             # 50176
HALF = 4 * BPC * BLK          # 25088
CHUNK_BLKS = 2
NLAYERS = 4

_cache = {}
BF16 = ml_dtypes.bfloat16


def _chunk_layout():
    chunks = []
    b = 0
    while b < BPC:
        chunks.append(list(range(b, min(b + CHUNK_BLKS, BPC))))
        b += CHUNK_BLKS
    return chunks


def _build_host(inputs):
    src = np.asarray(inputs['edge_index'][0], np.int64)
    dst = np.asarray(inputs['edge_index'][1], np.int64)
    lat = np.asarray(inputs['latency'], np.float32)

    order = np.argsort(dst, kind='stable')
    es, ed, el = src[order], dst[order], lat[order]
    blk_of = ed // BLK
    blk_starts = np.searchsorted(blk_of, np.arange(NBLK + 1))

    # per (core, block): lo/hi edge lists
    per = {}
    nlo = np.zeros((NCORES, BPC), np.int64)
    nhi = np.zeros((NCORES, BPC), np.int64)
    for k in range(NCORES):
        for b in range(BPC):
            g = k * BPC + b
            s_, e_ = blk_starts[g], blk_starts[g + 1]
            bs, bd, bl = es[s_:e_], ed[s_:e_] - g * BLK, el[s_:e_]
            lo = bs < HALF
            per[(k, b)] = (bs[lo], bd[lo], bl[lo], bs[~lo] - HALF, bd[~lo], bl[~lo])
            nlo[k, b] = len(bs[lo])
            nhi[k, b] = len(bs) - nlo[k, b]
    TLO = -(-nlo.max(axis=0) // 128)      # uniform tiles per block (lo half)
    THI = -(-nhi.max(axis=0) // 128)

    chunks = _chunk_layout()
    # global tile axis: chunk-major; within chunk: lo tiles (block-major), hi tiles
    tile_axis = []
    for blks in chunks:
        for b in blks:
            for t in range(int(TLO[b])):
                tile_axis.append(('lo', b, t))
        for b in blks:
            for t in range(int(THI[b])):
                tile_axis.append(('hi', b, t))
    ntiles = len(tile_axis)
    gi_of = {v: i for i, v in enumerate(tile_axis)}
    lo_order = [v for v in tile_axis if v[0] == 'lo']   # gather order, chunk-major
    hi_order = [v for v in tile_axis if v[0] == 'hi']
    lo_pos = {v: i for i, v in enumerate(lo_order)}
    hi_pos = {v: i for i, v in enumerate(hi_order)}
    n_lo_tiles, n_hi_tiles = len(lo_order), len(hi_order)

    idx_lo = np.zeros((NCORES, 128, n_lo_tiles * 8), np.int16)
    idx_hi = np.zeros((NCORES, 128, n_hi_tiles * 8), np.int16)
    dstcol = np.full((NCORES, 128, ntiles), float(BLK), np.float32)
    latcol = np.zeros((NCORES, 128, ntiles), np.float32)

    def wrap16(a):     # [128] int -> [128, 8] int16 wrapped+replicated
        return np.tile(a.astype(np.int16).reshape(-1, 16).T, (8, 1))

    for k in range(NCORES):
        for b in range(BPC):
            slo, dlo, llo, shi, dhi, lhi = per[(k, b)]
            for half, s_, d_, l_, T_, pos, idx_arr in (
                    ('lo', slo, dlo, llo, TLO, lo_pos, idx_lo),
                    ('hi', shi, dhi, lhi, THI, hi_pos, idx_hi)):
                nt = int(T_[b])
                if nt == 0:
                    continue
                cap = nt * 128
                sp = np.zeros(cap, np.int64)
                dp = np.full(cap, BLK, np.int64)
                lp = np.zeros(cap, np.float32)
                sp[:len(s_)] = s_
                dp[:len(d_)] = d_
                lp[:len(l_)] = l_
                for t in range(nt):
                    p = pos[(half, b, t)]
                    idx_arr[k][:, p * 8:(p + 1) * 8] = wrap16(sp[t * 128:(t + 1) * 128])
                    gi = gi_of[(half, b, t)]
                    dstcol[k][:, gi] = dp[t * 128:(t + 1) * 128]
                    latcol[k][:, gi] = lp[t * 128:(t + 1) * 128]

    # ---- features ----
    type_ids = np.asarray(inputs['type_ids'], np.int64)
    node = np.arange(NPAD)
    valid = node < N
    k_ = node // (BPC * BLK)
    b_ = (node % (BPC * BLK)) // BLK
    p_ = node % BLK
    onehot4T = np.zeros((NCORES, 4, BPC * BLK), np.float32)
    tid = np.full(NPAD, -1, np.int64)
    tid[:N] = type_ids
    for t in range(4):
        m = tid == t
        onehot4T[k_[m], t, b_[m] * BLK + p_[m]] = 1.0

    def wrapnode(x):   # [N] -> [128, NBLK]
        o = np.zeros(NPAD, np.float32)
        o[:N] = x
        return o.reshape(-1, 128).T.copy()

    req_w_full = wrapnode(np.asarray(inputs['requests'], np.float32))
    us_w_full = wrapnode(np.asarray(inputs['update_step'], np.float32))
    idx_node = np.arange(NPAD).reshape(-1, 128).T
    mask_ge15 = ((idx_node >= NL) & (idx_node < N)).astype(np.float32)
    mask_lt15 = (idx_node < NL).astype(np.float32)

    perms = []
    for k in range(NCORES):
        own = np.arange(k * BPC, (k + 1) * BPC)
        rest = np.array([c for c in range(NBLK) if not (k * BPC <= c < (k + 1) * BPC)])
        perms.append(np.concatenate([own, rest]))

    def rep_row(v):    # [HID] -> [128, HID]
        return np.tile(np.asarray(v, np.float32).reshape(1, -1), (128, 1))

    def we_fold(We, a_e):
        We = np.asarray(We, np.float32).reshape(1, HID)
        a_e = np.asarray(a_e, np.float32)
        return np.array([(We[0, h * C:(h + 1) * C] * a_e[h]).sum() for h in range(H)],
                        np.float32)

    W0 = np.asarray(inputs['W0'], np.float32)
    T0 = (np.asarray(inputs['emb'], np.float32) @ W0[:F]).astype(np.float32)

    Ls = []
    Ls.append(dict(a_s=np.asarray(inputs['as0'], np.float32).reshape(HID),
                   a_d=np.asarray(inputs['ad0'], np.float32).reshape(HID),
                   we=we_fold(inputs['We0'], inputs['ae0']),
                   b=np.asarray(inputs['b0'], np.float32),
                   Wn=np.asarray(inputs['Wh'][0], np.float32)))
    Ls.append(dict(a_s=np.asarray(inputs['ash'][0], np.float32).reshape(HID),
                   a_d=np.asarray(inputs['adh'][0], np.float32).reshape(HID),
                   we=we_fold(np.asarray(inputs['Weh'][0]).reshape(1, -1), inputs['aeh'][0]),
                   b=np.asarray(inputs['bh'][0], np.float32),
                   Wn=np.asarray(inputs['Wh'][1], np.float32)))
    Ls.append(dict(a_s=np.asarray(inputs['ash'][1], np.float32).reshape(HID),
                   a_d=np.asarray(inputs['adh'][1], np.float32).reshape(HID),
                   we=we_fold(np.asarray(inputs['Weh'][1]).reshape(1, -1), inputs['aeh'][1]),
                   b=np.asarray(inputs['bh'][1], np.float32),
                   Wn=np.asarray(inputs['Wf'], np.float32)))
    Ls.append(dict(a_s=np.asarray(inputs['asf'], np.float32).reshape(HID),
                   a_d=np.asarray(inputs['adf'], np.float32).reshape(HID),
                   we=we_fold(inputs['Wef'], inputs['aef']),
                   b=np.asarray(inputs['bf'], np.float32),
                   Wn=None))

    a_s_all = np.concatenate([rep_row(L['a_s']) for L in Ls], axis=1).astype(BF16)
    a_d_all = np.concatenate([rep_row(L['a_d']) for L in Ls], axis=1).astype(BF16)
    b_all = np.concatenate([rep_row(L['b']) for L in Ls], axis=1).astype(np.float32)
    we_all = np.concatenate([np.tile(L['we'].reshape(1, H), (128, 1)) for L in Ls],
                            axis=1).astype(np.float32)
    Wn_all = np.concatenate([Ls[i]['Wn'] for i in range(3)], axis=1).astype(BF16)

    batch = np.asarray(inputs['batch'], np.int64)
    cnt = np.zeros(NG, np.float64)
    np.add.at(cnt, batch, 1.0)
    pool_mat = np.zeros((NCORES, 128, BPC * NG), np.float32)
    bv = batch[node[valid]]
    pool_mat[k_[valid], p_[valid], b_[valid] * NG + bv] = 1.0

    C2w = np.asarray(inputs['C2w'], np.float32)
    host = dict(
        TLO=TLO, THI=THI, chunks=chunks, ntiles=ntiles,
        n_lo_tiles=n_lo_tiles, n_hi_tiles=n_hi_tiles,
        idx_lo=idx_lo, idx_hi=idx_hi, dstcol=dstcol.astype(BF16),
        latcol=latcol,
        onehot4T=onehot4T.astype(BF16), req_w_full=req_w_full, us_w_full=us_w_full,
        mask_ge15=mask_ge15, mask_lt15=mask_lt15, perms=perms,
        T0=T0.astype(BF16),
        w16_rep=rep_row(np.tile(W0[F], 1)), w17_rep=rep_row(W0[F + 1]),
        a_s_all=a_s_all, a_d_all=a_d_all, b_all=b_all, we_all=we_all,
        Wn_all=Wn_all, cnt=cnt, pool_mat=pool_mat.astype(BF16),
        C1w=np.asarray(inputs['C1w'], np.float32).astype(BF16),
        C2w=np.concatenate([C2w[0:128], C2w[128:256]], axis=1).astype(BF16),
        C3w=np.ascontiguousarray(np.asarray(inputs['C3w'], np.float32).reshape(2, 128).T).astype(BF16),
        c1b_col=np.ascontiguousarray(np.asarray(inputs['C1b'], np.float32).reshape(2, 128).T),
        c2b_col=np.ascontiguousarray(np.asarray(inputs['C2b'], np.float32).reshape(2, 128).T),
        C3b=float(np.asarray(inputs['C3b'], np.float32)[0]),
        iota_row_bf=np.tile(np.arange(128, dtype=np.float32)[None, :], (128, 1)).astype(BF16),
        ident_bf=np.eye(128, dtype=np.float32).astype(BF16),
        ones_col=np.ones((128, 1), np.float32),
    )
    return host


def _build_program(host):
    import concourse.bacc as bacc
    import concourse.mybir as mybir
    import concourse.tile as tile
    from concourse import library_config
    F32 = mybir.dt.float32
    BF = mybir.dt.bfloat16
    I16 = mybir.dt.int16
    ALU = mybir.AluOpType
    AX = mybir.AxisListType
    ACTF = mybir.ActivationFunctionType

    TLO, THI, chunks = host['TLO'], host['THI'], host['chunks']
    ntiles = host['ntiles']
    nlo, nhi = host['n_lo_tiles'], host['n_hi_tiles']

    nc = bacc.Bacc("TRN2", target_bir_lowering=False, debug=False, num_devices=NCORES)

    ins = {
        'idx_lo': ([128, nlo * 8], I16), 'idx_hi': ([128, nhi * 8], I16),
        'dstcol': ([128, ntiles], BF), 'latcol': ([128, ntiles], F32),
        'a_s_all': ([128, NLAYERS * HID], BF), 'a_d_all': ([128, NLAYERS * HID], BF),
        'b_all': ([128, NLAYERS * HID], F32), 'we_all': ([128, NLAYERS * H], F32),
        'Wn_all': ([128, 3 * HID], BF),
        'req_w': ([128, NBLK], F32), 'us_own': ([128, BPC], F32),
        'mask_ge15': ([128, NBLK], F32), 'mask_lt15': ([128, NBLK], F32),
        'onehot4T': ([4, BPC * BLK], BF), 'T0': ([4, HID], BF),
        'w16_rep': ([128, HID], F32), 'w17_rep': ([128, HID], F32),
        'C1w': ([HID, FC], BF), 'C2w': ([128, 2 * FC], BF), 'C3w': ([128, 2], BF),
        'c1b_col': ([128, 2], F32), 'c2b_col': ([128, 2], F32),
        'pool_mat': ([128, BPC * NG], BF),
        'iota_row_bf': ([128, 128], BF), 'ident_bf': ([128, 128], BF),
        'ones_col': ([128, 1], F32),
    }
    tin = {}
    for name, (shape, dt) in ins.items():
        tin[name] = nc.dram_tensor(name, list(shape), dt, kind="ExternalInput")
    t_part = nc.dram_tensor('partials', [NG, 1], F32, kind="ExternalOutput")

    # static chunk metadata: per chunk -> (lo_tile_start, glo, hi_tile_start, ghi,
    #   per-block slot lists, global tile index base)
    chunk_meta = []
    lo_base = hi_base = gi_base = 0
    for blks in chunks:
        glo = int(sum(TLO[b] for b in blks))
        ghi = int(sum(THI[b] for b in blks))
        T = glo + ghi
        # chunk slot s in [0,T): lo slots first (block-major), then hi
        per_block = []     # (b, [slots], [is_lo flags])
        s = 0
        slots_lo = {}
        for b in blks:
            slots_lo[b] = list(range(s, s + int(TLO[b])))
            s += int(TLO[b])
        slots_hi = {}
        for b in blks:
            slots_hi[b] = list(range(s, s + int(THI[b])))
            s += int(THI[b])
        for b in blks:
            per_block.append((b, slots_lo[b] + slots_hi[b]))
        chunk_meta.append(dict(blks=blks, glo=glo, ghi=ghi, T=T,
                               lo0=lo_base, hi0=hi_base, gi0=gi_base,
                               per_block=per_block))
        lo_base += glo
        hi_base += ghi
        gi_base += T

    with tile.TileContext(nc) as tc:
        with (
            tc.tile_pool(name="const", bufs=1) as constp,
            tc.tile_pool(name="gbuf", bufs=2) as gbufp,
            tc.tile_pool(name="ohp", bufs=2) as ohp,
            tc.tile_pool(name="wsp", bufs=2) as wsp,
            tc.tile_pool(name="work", bufs=2) as workp,
            tc.tile_pool(name="slice", bufs=2) as slicep,
            tc.tile_pool(name="psT", bufs=2, space="PSUM") as psT,
            tc.tile_pool(name="psE", bufs=3, space="PSUM") as psE,
            tc.tile_pool(name="psG", bufs=1, space="PSUM") as psG,
            tc.tile_pool(name="psMM", bufs=2, space="PSUM") as psMM,
            tc.tile_pool(name="dram", bufs=2, space="DRAM") as dramp,
            tc.tile_pool(name="dramO", bufs=1, space="DRAM") as dramO,
        ):
            nc.gpsimd.load_library(library_config.mlp)
            # layer-independent transposed one-hot matrices, built in layer 0
            ohT_dram = dramO.tile([ntiles * 128, 128], BF, tag="ohT_dram", name="ohT_dram")
            c = {}
            for name in ins:
                shape, dt = ins[name]
                t = constp.tile(list(shape), dt, tag=name, name=name)
                nc.sync.dma_start(t[:], tin[name].ap())
                c[name] = t

            # ---------------- feat phase: xp1 own slice ----------------
            n = float(N - NL)
            d = workp.tile([128, NBLK], F32, tag="fd", name="fd")
            nc.vector.tensor_tensor(out=d[:], in0=c['req_w'][:], in1=c['mask_ge15'][:], op=ALU.mult)
            col = workp.tile([128, 1], F32, tag="fcol", name="fcol")
            nc.vector.tensor_reduce(out=col[:], in_=d[:], op=ALU.add, axis=AX.X)
            tot = psMM.tile([1, 1], F32, tag="mm", name="ftot")
            nc.tensor.matmul(tot[:], col[:], c['ones_col'][:], start=True, stop=True)
            mean = workp.tile([1, 1], F32, tag="fmean", name="fmean")
            nc.vector.tensor_scalar(out=mean[:], in0=tot[:], scalar1=1.0 / n, scalar2=None, op0=ALU.mult)
            mean_col = workp.tile([128, 1], F32, tag="fmc", name="fmc")
            nc.gpsimd.partition_broadcast(mean_col[:], mean[:])
            nc.vector.tensor_scalar(out=d[:], in0=c['req_w'][:], scalar1=mean_col[:, 0:1], scalar2=None, op0=ALU.subtract)
            nc.vector.tensor_tensor(out=d[:], in0=d[:], in1=c['mask_ge15'][:], op=ALU.mult)
            d2 = workp.tile([128, NBLK], F32, tag="fd2", name="fd2")
            nc.vector.tensor_tensor(out=d2[:], in0=d[:], in1=d[:], op=ALU.mult)
            nc.vector.tensor_reduce(out=col[:], in_=d2[:], op=ALU.add, axis=AX.X)
            tot2 = psMM.tile([1, 1], F32, tag="mm", name="ftot2")
            nc.tensor.matmul(tot2[:], col[:], c['ones_col'][:], start=True, stop=True)
            var = workp.tile([1, 1], F32, tag="fvar", name="fvar")
            nc.vector.tensor_scalar(out=var[:], in0=tot2[:], scalar1=1.0 / (n - 1.0), scalar2=None, op0=ALU.mult)
            std = workp.tile([1, 1], F32, tag="fstd", name="fstd")
            nc.scalar.activation(out=std[:], in_=var[:], func=ACTF.Sqrt)
            nc.vector.tensor_scalar(out=std[:], in0=std[:], scalar1=1e-6, scalar2=None, op0=ALU.add)
            rinv = workp.tile([1, 1], F32, tag="frinv", name="frinv")
            nc.vector.reciprocal(out=rinv[:], in_=std[:])
            rinv_col = workp.tile([128, 1], F32, tag="frc", name="frc")
            nc.gpsimd.partition_broadcast(rinv_col[:], rinv[:])
            rf = workp.tile([128, NBLK], F32, tag="frf", name="frf")
            nc.vector.tensor_scalar(out=rf[:], in0=d[:], scalar1=rinv_col[:, 0:1], scalar2=None, op0=ALU.mult)
            raw15 = workp.tile([128, NBLK], F32, tag="fr15", name="fr15")
            nc.vector.tensor_tensor(out=raw15[:], in0=c['req_w'][:], in1=c['mask_lt15'][:], op=ALU.mult)
            nc.vector.tensor_tensor(out=rf[:], in0=rf[:], in1=raw15[:], op=ALU.add)

            xpown = slicep.tile([128, BPC, HID], BF, tag="xpown", name="xpown0")
            for b in range(BPC):
                mm = psMM.tile([128, HID], F32, tag="mm", name="fmm")
                nc.tensor.matmul(mm[:], c['onehot4T'][:, b * 128:(b + 1) * 128], c['T0'][:],
                                 start=True, stop=True)
                x0 = workp.tile([128, HID], F32, tag="fx0", name="fx0")
                t1 = workp.tile([128, HID], F32, tag="ft1", name="ft1")
                nc.vector.tensor_scalar(out=t1[:], in0=c['w16_rep'][:], scalar1=rf[:, b:b + 1], scalar2=None, op0=ALU.mult)
                nc.vector.tensor_tensor(out=x0[:], in0=mm[:], in1=t1[:], op=ALU.add)
                nc.vector.tensor_scalar(out=t1[:], in0=c['w17_rep'][:], scalar1=c['us_own'][:, b:b + 1], scalar2=None, op0=ALU.mult)
                nc.vector.tensor_tensor(out=x0[:], in0=x0[:], in1=t1[:], op=ALU.add)
                nc.scalar.copy(out=xpown[:, b, :], in_=x0[:])

            # ---------------- 4 GAT layers ----------------
            for li in range(NLAYERS):
                a_s = c['a_s_all'][:, li * HID:(li + 1) * HID]
                a_d = c['a_d_all'][:, li * HID:(li + 1) * HID]
                b_rep = c['b_all'][:, li * HID:(li + 1) * HID]
                we_rep = c['we_all'][:, li * H:(li + 1) * H]
                relu = li < NLAYERS - 1

                # sdst per own node (bf16)
                xd = workp.tile([128, BPC, HID], BF, tag="xd", name=f"xd{li}", bufs=1)
                nc.vector.tensor_tensor(
                    out=xd[:], in0=xpown[:],
                    in1=a_d.rearrange("p f -> p () f").broadcast_to([128, BPC, HID]),
                    op=ALU.mult)
                sdf = workp.tile([128, BPC, H], F32, tag="sdf", name=f"sdf{li}")
                nc.vector.tensor_reduce(
                    out=sdf[:], in_=xd[:].rearrange("p b (h c) -> p b h c", h=H),
                    op=ALU.add, axis=AX.X)
                sdst = workp.tile([128, BPC, H], BF, tag="sdst", name=f"sdst{li}")
                nc.vector.tensor_scalar(out=sdst[:], in0=sdf[:], scalar1=0.0, scalar2=None, op0=ALU.add)

                # ship own xp slice, AllGather full table
                xsl = dramp.tile([BPC * BLK, HID], BF, tag="xsl", name=f"xsl{li}")
                nc.sync.dma_start(xsl[:].rearrange("(b p) j -> p b j", p=128), xpown[:])
                tab = dramp.tile([NPAD, HID], BF, tag="tab", addr_space="Shared", name=f"tab{li}")
                nc.gpsimd.collective_compute(
                    "AllGather", ALU.bypass,
                    replica_groups=[list(range(NCORES))],
                    ins=[xsl[:]], outs=[tab[:]])

                xslice = slicep.tile([128, BPC, HID], BF, tag="xslice", name=f"xslice{li}", bufs=1)

                for cm in chunk_meta:
                    glo, ghi, T = cm['glo'], cm['ghi'], cm['T']
                    g_lo = gbufp.tile([128, max(glo, 1), HID], BF, tag="g_lo", name=f"glo{li}_{cm['gi0']}")
                    g_hi = gbufp.tile([128, max(ghi, 1), HID], BF, tag="g_hi", name=f"ghi{li}_{cm['gi0']}")
                    if glo:
                        nc.gpsimd.dma_gather(
                            g_lo[:, 0:glo, :], tab[0:HALF, :],
                            c['idx_lo'][:, cm['lo0'] * 8:(cm['lo0'] + glo) * 8],
                            glo * 128, glo * 128, HID, single_packet=False)
                    if ghi:
                        nc.gpsimd.dma_gather(
                            g_hi[:, 0:ghi, :], tab[HALF:NPAD, :],
                            c['idx_hi'][:, cm['hi0'] * 8:(cm['hi0'] + ghi) * 8],
                            ghi * 128, ghi * 128, HID, single_packet=False)

                    # s_src for all chunk slots (lo block-major, then hi)
                    s_src = workp.tile([128, T, H], F32, tag="s_src", name=f"ss{li}_{cm['gi0']}")
                    xa = workp.tile([128, T, HID], BF, tag="xa", name=f"xa{li}_{cm['gi0']}", bufs=1)
                    if glo:
                        nc.vector.tensor_tensor(
                            out=xa[:, 0:glo, :], in0=g_lo[:, 0:glo, :],
                            in1=a_s.rearrange("p f -> p () f").broadcast_to([128, glo, HID]),
                            op=ALU.mult)
                    if ghi:
                        nc.vector.tensor_tensor(
                            out=xa[:, glo:T, :], in0=g_hi[:, 0:ghi, :],
                            in1=a_s.rearrange("p f -> p () f").broadcast_to([128, ghi, HID]),
                            op=ALU.mult)
                    nc.vector.tensor_reduce(
                        out=s_src[:], in_=xa[:].rearrange("p t (h c) -> p t h c", h=H),
                        op=ALU.add, axis=AX.X)

                    # one-hot dst matrices for the chunk
                    oh_ch = ohp.tile([128, T, 128], BF, tag="oh_ch", name=f"oh{li}_{cm['gi0']}")
                    nc.vector.tensor_tensor(
                        out=oh_ch[:],
                        in0=c['iota_row_bf'][:].rearrange("p f -> p () f").broadcast_to([128, T, 128]),
                        in1=c['dstcol'][:, cm['gi0']:cm['gi0'] + T].rearrange("p t -> p t ()").broadcast_to([128, T, 128]),
                        op=ALU.is_equal)

                    # s_dst per edge: ohT (layer-independent) built in layer 0,
                    # cached in DRAM for layers 1-3; sp matmul against sdst
                    ohT_ch = ohp.tile([128, T, 128], BF, tag="ohT_ch", name=f"ohTc{li}_{cm['gi0']}")
                    od = ohT_dram[cm['gi0'] * 128:(cm['gi0'] + T) * 128, :]
                    if li == 0:
                        for t0 in range(0, T, 4):
                            tn = min(4, T - t0)
                            tp = psT.tile([128, 4, 128], BF, tag="tp", name=f"tp{cm['gi0']}_{t0}")
                            for i in range(tn):
                                nc.tensor.transpose(tp[:, i, :], oh_ch[:, t0 + i, :], c['ident_bf'][:])
                            nc.scalar.copy(out=ohT_ch[:, t0:t0 + tn, :], in_=tp[:, 0:tn, :])
                        nc.sync.dma_start(od.rearrange("(t j) f -> j t f", j=128), ohT_ch[:])
                    else:
                        nc.sync.dma_start(ohT_ch[:], od.rearrange("(t j) f -> j t f", j=128))
                    sp_ps = psE.tile([128, T, H], F32, tag="eacc", name=f"spps{li}_{cm['gi0']}")
                    for b, slots in cm['per_block']:
                        for t in slots:
                            nc.tensor.matmul(sp_ps[:, t, :], ohT_ch[:, t, :], sdst[:, b, :],
                                             start=True, stop=True)

                    # alpha: s_src + s_dst + we*lat; leaky_relu; exp
                    latw = workp.tile([128, T, H], F32, tag="latw", name=f"lw{li}_{cm['gi0']}")
                    nc.vector.tensor_tensor(
                        out=latw[:],
                        in0=c['latcol'][:, cm['gi0']:cm['gi0'] + T].rearrange("p t -> p t ()").broadcast_to([128, T, H]),
                        in1=we_rep.rearrange("p h -> p () h").broadcast_to([128, T, H]),
                        op=ALU.mult)
                    araw = workp.tile([128, T, H], F32, tag="araw", name=f"ar{li}_{cm['gi0']}")
                    nc.vector.tensor_tensor(out=araw[:], in0=s_src[:], in1=sp_ps[:], op=ALU.add)
                    nc.vector.tensor_tensor(out=araw[:], in0=araw[:], in1=latw[:], op=ALU.add)
                    lr = workp.tile([128, T, H], F32, tag="lr", name=f"lr{li}_{cm['gi0']}")
                    nc.vector.tensor_scalar(out=lr[:], in0=araw[:], scalar1=0.2, scalar2=None, op0=ALU.mult)
                    nc.vector.tensor_tensor(out=araw[:], in0=araw[:], in1=lr[:], op=ALU.max)

                    wstack = wsp.tile([128, T, H + HID], BF, tag="wstack", name=f"wst{li}_{cm['gi0']}")
                    wexp = workp.tile([128, T, H], BF, tag="wexp", name=f"we{li}_{cm['gi0']}")
                    nc.scalar.activation(out=wexp[:], in_=araw[:], func=ACTF.Exp)
                    nc.scalar.activation(out=wstack[:, :, 0:H], in_=araw[:], func=ACTF.Exp)
                    # weighted messages from raw gathered features
                    if glo:
                        nc.vector.tensor_tensor(
                            out=wstack[:, 0:glo, H:].rearrange("p t (h c) -> p t h c", h=H),
                            in0=g_lo[:, 0:glo, :].rearrange("p t (h c) -> p t h c", h=H),
                            in1=wexp[:, 0:glo, :].rearrange("p t h -> p t h ()").broadcast_to([128, glo, H, C]),
                            op=ALU.mult)
                    if ghi:
                        nc.vector.tensor_tensor(
                            out=wstack[:, glo:T, H:].rearrange("p t (h c) -> p t h c", h=H),
                            in0=g_hi[:, 0:ghi, :].rearrange("p t (h c) -> p t h c", h=H),
                            in1=wexp[:, glo:T, :].rearrange("p t h -> p t h ()").broadcast_to([128, ghi, H, C]),
                            op=ALU.mult)

                    # scatter per dst block: [den | num] accumulated on PE
                    for b, slots in cm['per_block']:
                        acc = psE.tile([128, H + HID], F32, tag="eacc", name=f"acc{li}_{b}")
                        for j, t in enumerate(slots):
                            nc.tensor.matmul(acc[:], oh_ch[:, t, :], wstack[:, t, :],
                                             start=(j == 0), stop=(j == len(slots) - 1))
                        den = workp.tile([128, H], F32, tag="den", name=f"den{li}_{b}")
                        nc.vector.tensor_scalar(out=den[:], in0=acc[:, 0:H], scalar1=1e-16, scalar2=None, op0=ALU.add)
                        recip = workp.tile([128, H], F32, tag="recip", name=f"rc{li}_{b}")
                        nc.vector.reciprocal(out=recip[:], in_=den[:])
                        xn = workp.tile([128, HID], F32, tag="xn", name=f"xn{li}_{b}")
                        nc.vector.tensor_tensor(
                            out=xn[:].rearrange("p (h c) -> p h c", h=H),
                            in0=acc[:, H:].rearrange("p (h c) -> p h c", h=H),
                            in1=recip[:].rearrange("p h -> p h ()").broadcast_to([128, H, C]),
                            op=ALU.mult)
                        nc.vector.tensor_tensor(out=xn[:], in0=xn[:], in1=b_rep, op=ALU.add)
                        if relu:
                            nc.scalar.activation(out=xslice[:, b, :], in_=xn[:], func=ACTF.Relu)
                        else:
                            nc.scalar.copy(out=xslice[:, b, :], in_=xn[:])

                if li < NLAYERS - 1:
                    Wn = c['Wn_all'][:, li * HID:(li + 1) * HID]
                    xpown = slicep.tile([128, BPC, HID], BF, tag="xpown", name=f"xpown{li + 1}")
                    for b in range(BPC):
                        tp = psT.tile([128, 128], BF, tag="tp", name=f"ntp{li}_{b}")
                        nc.tensor.transpose(tp[:], xslice[:, b, :], c['ident_bf'][:])
                        xT = workp.tile([128, 128], BF, tag="xT", name=f"nxT{li}_{b}")
                        nc.scalar.copy(out=xT[:], in_=tp[:])
                        xpp = psMM.tile([128, HID], F32, tag="mm", name=f"nxpp{li}_{b}")
                        nc.tensor.matmul(xpp[:], xT[:], Wn, start=True, stop=True)
                        nc.scalar.copy(out=xpown[:, b, :], in_=xpp[:])

            # ---------------- MLP head + pool ----------------
            gp = psG.tile([NG, 1], F32, tag="gp", name="gp")
            for b in range(BPC):
                tp = psT.tile([128, 128], BF, tag="tp", name=f"mtp{b}")
                nc.tensor.transpose(tp[:], xslice[:, b, :], c['ident_bf'][:])
                xT = workp.tile([128, 128], BF, tag="xT", name=f"mxT{b}")
                nc.scalar.copy(out=xT[:], in_=tp[:])
                h1 = []
                for jh in range(2):
                    hp = psMM.tile([128, 128], F32, tag="mm", name=f"mh1p{b}_{jh}")
                    nc.tensor.matmul(hp[:], c['C1w'][:, jh * 128:(jh + 1) * 128], xT[:],
                                     start=True, stop=True)
                    hs = workp.tile([128, 128], BF, tag=f"h1_{jh}", name=f"mh1s{b}_{jh}")
                    nc.vector.tensor_scalar(out=hs[:], in0=hp[:],
                                            scalar1=c['c1b_col'][:, jh:jh + 1],
                                            scalar2=0.0, op0=ALU.add, op1=ALU.max)
                    h1.append(hs)
                h2 = []
                for jh in range(2):
                    hp = psMM.tile([128, 128], F32, tag="mm", name=f"mh2p{b}_{jh}")
                    for kc in range(2):
                        nc.tensor.matmul(hp[:], c['C2w'][:, kc * FC + jh * 128:kc * FC + (jh + 1) * 128],
                                         h1[kc][:], start=(kc == 0), stop=(kc == 1))
                    hs = workp.tile([128, 128], BF, tag=f"h2_{jh}", name=f"mh2s{b}_{jh}")
                    nc.vector.tensor_scalar(out=hs[:], in0=hp[:],
                                            scalar1=c['c2b_col'][:, jh:jh + 1],
                                            scalar2=0.0, op0=ALU.add, op1=ALU.max)
                    h2.append(hs)
                nvp = psMM.tile([128, 1], F32, tag="mm", name=f"mnvp{b}")
                for kc in range(2):
                    nc.tensor.matmul(nvp[:], h2[kc][:], c['C3w'][:, kc:kc + 1],
                                     start=(kc == 0), stop=(kc == 1))
                nv = workp.tile([128, 1], BF, tag="nv", name=f"mnv{b}")
                nc.vector.tensor_scalar(out=nv[:], in0=nvp[:], scalar1=host['C3b'],
                                        scalar2=0.0, op0=ALU.add, op1=ALU.max)
                nc.tensor.matmul(gp[:], c['pool_mat'][:, b * NG:(b + 1) * NG], nv[:],
                                 start=(b == 0), stop=(b == BPC - 1))
            pt = workp.tile([NG, 1], F32, tag="pt", name="pt")
            nc.scalar.copy(out=pt[:], in_=gp[:])
            nc.sync.dma_start(t_part.ap(), pt[:])

    nc.compile()
    return nc


def kernel(**inputs):
    from concourse.bass_utils import run_bass_kernel_spmd
    import hashlib
    inputs = {k: np.asarray(v) for k, v in inputs.items()}
    host = _build_host(inputs)
    key = hashlib.sha1(np.ascontiguousarray(inputs['edge_index']).tobytes()).hexdigest()
    if key not in _cache:
        _cache[key] = _build_program(host)
    prog = _cache[key]

    in_maps = []
    for k in range(NCORES):
        perm = host['perms'][k]
        in_maps.append(dict(
            idx_lo=host['idx_lo'][k], idx_hi=host['idx_hi'][k],
            dstcol=host['dstcol'][k], latcol=host['latcol'][k],
            a_s_all=host['a_s_all'], a_d_all=host['a_d_all'],
            b_all=host['b_all'], we_all=host['we_all'], Wn_all=host['Wn_all'],
            req_w=np.ascontiguousarray(host['req_w_full'][:, perm]),
            us_own=np.ascontiguousarray(host['us_w_full'][:, k * BPC:(k + 1) * BPC]),
            mask_ge15=np.ascontiguousarray(host['mask_ge15'][:, perm]),
            mask_lt15=np.ascontiguousarray(host['mask_lt15'][:, perm]),
            onehot4T=host['onehot4T'][k], T0=host['T0'],
            w16_rep=host['w16_rep'], w17_rep=host['w17_rep'],
            C1w=host['C1w'], C2w=host['C2w'], C3w=host['C3w'],
            c1b_col=host['c1b_col'], c2b_col=host['c2b_col'],
            pool_mat=host['pool_mat'][k],
            iota_row_bf=host['iota_row_bf'], ident_bf=host['ident_bf'],
            ones_col=host['ones_col'],
        ))
    def _launch():
        t0 = time.monotonic()
        res = run_bass_kernel_spmd(prog, in_maps, core_ids=list(range(NCORES)))
        wall = (time.monotonic() - t0) * 1e9
        t = res.exec_time_ns if res.exec_time_ns else wall
        p = sum(np.asarray(res.results[k]['partials'], np.float64) for k in range(NCORES))
        return p, t

    # The axon terminal occasionally returns corrupted results right after a
    # device reset; run twice (second launch is cheap in-process) and verify.
    times = []
    p1, t = _launch()
    times.append(t)
    p2, t = _launch()
    times.append(t)
    if not np.allclose(p1, p2, rtol=1e-3, atol=1e-6):
        p3, t = _launch()
        times.append(t)
        if np.allclose(p2, p3, rtol=1e-3, atol=1e-6):
            p1 = p2
        elif np.allclose(p1, p3, rtol=1e-3, atol=1e-6):
            pass
        else:
            p1 = p3
    partials = p1
    out = (partials[:, 0] / np.maximum(host['cnt'], 1.0)).astype(np.float32)[:, None]
    kernel._last_times = times
    return out


# revision 4
# speedup vs baseline: 1.3355x; 1.3355x over previous
"""CriticSwapGNN Trainium2 fused kernel: feat + 4 GAT layers + MLP head + pool
in ONE launch across 8 cores, with on-device AllGather of xp between layers.

Sharding: dst-range ownership, 8 cores x 49 blocks of 128 nodes. Edges sorted
by dst block, split lo/hi by src half (int16 gather indices), tiled 128/tile,
grouped in chunks of CHUNK_BLKS dst blocks. Per chunk: dma_gather of src rows
from the AllGathered table plus dst rows from the core-local slice (s_src and
s_dst both computed per-edge on DVE), segment softmax without max-subtraction
(logits are bounded), one fused [den|num] scatter matmul per tile into PSUM.
Table and matmul operands in bf16 (rel err ~4.5e-3 vs 2e-2 gate); accumulations
in fp32 PSUM. The launch is run twice and cross-checked (the axon terminal
occasionally returns corrupted results right after a device reset).
"""
import sys
import time
import numpy as np
import ml_dtypes

if '/opt/trn_rl_repo' not in sys.path:
    sys.path.insert(0, '/opt/trn_rl_repo')

N = 50000; E = 800000; F = 16; HID = 128; H = 4; C = 32; FC = 256; NL = 15; NG = 8
NCORES = 8
BLK = 128
BPC = 49                      # blocks per core
NBLK = NCORES * BPC           # 392
NPAD = NBLK * BLK             # 50176
HALF = 4 * BPC * BLK          # 25088
CHUNK_BLKS = 2
NLAYERS = 4

_cache = {}
BF16 = ml_dtypes.bfloat16


def _chunk_layout():
    chunks = []
    b = 0
    while b < BPC:
        chunks.append(list(range(b, min(b + CHUNK_BLKS, BPC))))
        b += CHUNK_BLKS
    return chunks


def _build_host(inputs):
    src = np.asarray(inputs['edge_index'][0], np.int64)
    dst = np.asarray(inputs['edge_index'][1], np.int64)
    lat = np.asarray(inputs['latency'], np.float32)

    order = np.argsort(dst, kind='stable')
    es, ed, el = src[order], dst[order], lat[order]
    blk_of = ed // BLK
    blk_starts = np.searchsorted(blk_of, np.arange(NBLK + 1))

    # per (core, block): lo/hi edge lists
    per = {}
    nlo = np.zeros((NCORES, BPC), np.int64)
    nhi = np.zeros((NCORES, BPC), np.int64)
    for k in range(NCORES):
        for b in range(BPC):
            g = k * BPC + b
            s_, e_ = blk_starts[g], blk_starts[g + 1]
            bs, bd, bl = es[s_:e_], ed[s_:e_] - g * BLK, el[s_:e_]
            lo = bs < HALF
            per[(k, b)] = (bs[lo], bd[lo], bl[lo], bs[~lo] - HALF, bd[~lo], bl[~lo])
            nlo[k, b] = len(bs[lo])
            nhi[k, b] = len(bs) - nlo[k, b]
    TLO = -(-nlo.max(axis=0) // 128)      # uniform tiles per block (lo half)
    THI = -(-nhi.max(axis=0) // 128)

    chunks = _chunk_layout()
    # global tile axis: chunk-major; within chunk: lo tiles (block-major), hi tiles
    tile_axis = []
    for blks in chunks:
        for b in blks:
            for t in range(int(TLO[b])):
                tile_axis.append(('lo', b, t))
        for b in blks:
            for t in range(int(THI[b])):
                tile_axis.append(('hi', b, t))
    ntiles = len(tile_axis)
    gi_of = {v: i for i, v in enumerate(tile_axis)}
    lo_order = [v for v in tile_axis if v[0] == 'lo']   # gather order, chunk-major
    hi_order = [v for v in tile_axis if v[0] == 'hi']
    lo_pos = {v: i for i, v in enumerate(lo_order)}
    hi_pos = {v: i for i, v in enumerate(hi_order)}
    n_lo_tiles, n_hi_tiles = len(lo_order), len(hi_order)

    idx_lo = np.zeros((NCORES, 128, n_lo_tiles * 8), np.int16)
    idx_hi = np.zeros((NCORES, 128, n_hi_tiles * 8), np.int16)
    idx_dst = np.zeros((NCORES, 128, ntiles * 8), np.int16)
    dstcol = np.full((NCORES, 128, ntiles), float(BLK), np.float32)
    latcol = np.zeros((NCORES, 128, ntiles), np.float32)

    def wrap16(a):     # [128] int -> [128, 8] int16 wrapped+replicated
        return np.tile(a.astype(np.int16).reshape(-1, 16).T, (8, 1))

    for k in range(NCORES):
        for b in range(BPC):
            slo, dlo, llo, shi, dhi, lhi = per[(k, b)]
            for half, s_, d_, l_, T_, pos, idx_arr in (
                    ('lo', slo, dlo, llo, TLO, lo_pos, idx_lo),
                    ('hi', shi, dhi, lhi, THI, hi_pos, idx_hi)):
                nt = int(T_[b])
                if nt == 0:
                    continue
                cap = nt * 128
                sp = np.zeros(cap, np.int64)
                dp = np.full(cap, BLK, np.int64)
                dl = np.zeros(cap, np.int64)        # dst local node (pad -> 0)
                lp = np.zeros(cap, np.float32)
                sp[:len(s_)] = s_
                dp[:len(d_)] = d_
                dl[:len(d_)] = b * BLK + d_
                lp[:len(l_)] = l_
                for t in range(nt):
                    p = pos[(half, b, t)]
                    idx_arr[k][:, p * 8:(p + 1) * 8] = wrap16(sp[t * 128:(t + 1) * 128])
                    gi = gi_of[(half, b, t)]
                    idx_dst[k][:, gi * 8:(gi + 1) * 8] = wrap16(dl[t * 128:(t + 1) * 128])
                    dstcol[k][:, gi] = dp[t * 128:(t + 1) * 128]
                    latcol[k][:, gi] = lp[t * 128:(t + 1) * 128]

    # ---- features ----
    type_ids = np.asarray(inputs['type_ids'], np.int64)
    node = np.arange(NPAD)
    valid = node < N
    k_ = node // (BPC * BLK)
    b_ = (node % (BPC * BLK)) // BLK
    p_ = node % BLK
    onehot4T = np.zeros((NCORES, 4, BPC * BLK), np.float32)
    tid = np.full(NPAD, -1, np.int64)
    tid[:N] = type_ids
    for t in range(4):
        m = tid == t
        onehot4T[k_[m], t, b_[m] * BLK + p_[m]] = 1.0

    def wrapnode(x):   # [N] -> [128, NBLK]
        o = np.zeros(NPAD, np.float32)
        o[:N] = x
        return o.reshape(-1, 128).T.copy()

    req_w_full = wrapnode(np.asarray(inputs['requests'], np.float32))
    us_w_full = wrapnode(np.asarray(inputs['update_step'], np.float32))
    idx_node = np.arange(NPAD).reshape(-1, 128).T
    mask_ge15 = ((idx_node >= NL) & (idx_node < N)).astype(np.float32)
    mask_lt15 = (idx_node < NL).astype(np.float32)

    perms = []
    for k in range(NCORES):
        own = np.arange(k * BPC, (k + 1) * BPC)
        rest = np.array([c for c in range(NBLK) if not (k * BPC <= c < (k + 1) * BPC)])
        perms.append(np.concatenate([own, rest]))

    def rep_row(v):    # [HID] -> [128, HID]
        return np.tile(np.asarray(v, np.float32).reshape(1, -1), (128, 1))

    def we_fold(We, a_e):
        We = np.asarray(We, np.float32).reshape(1, HID)
        a_e = np.asarray(a_e, np.float32)
        return np.array([(We[0, h * C:(h + 1) * C] * a_e[h]).sum() for h in range(H)],
                        np.float32)

    W0 = np.asarray(inputs['W0'], np.float32)
    T0 = (np.asarray(inputs['emb'], np.float32) @ W0[:F]).astype(np.float32)

    Ls = []
    Ls.append(dict(a_s=np.asarray(inputs['as0'], np.float32).reshape(HID),
                   a_d=np.asarray(inputs['ad0'], np.float32).reshape(HID),
                   we=we_fold(inputs['We0'], inputs['ae0']),
                   b=np.asarray(inputs['b0'], np.float32),
                   Wn=np.asarray(inputs['Wh'][0], np.float32)))
    Ls.append(dict(a_s=np.asarray(inputs['ash'][0], np.float32).reshape(HID),
                   a_d=np.asarray(inputs['adh'][0], np.float32).reshape(HID),
                   we=we_fold(np.asarray(inputs['Weh'][0]).reshape(1, -1), inputs['aeh'][0]),
                   b=np.asarray(inputs['bh'][0], np.float32),
                   Wn=np.asarray(inputs['Wh'][1], np.float32)))
    Ls.append(dict(a_s=np.asarray(inputs['ash'][1], np.float32).reshape(HID),
                   a_d=np.asarray(inputs['adh'][1], np.float32).reshape(HID),
                   we=we_fold(np.asarray(inputs['Weh'][1]).reshape(1, -1), inputs['aeh'][1]),
                   b=np.asarray(inputs['bh'][1], np.float32),
                   Wn=np.asarray(inputs['Wf'], np.float32)))
    Ls.append(dict(a_s=np.asarray(inputs['asf'], np.float32).reshape(HID),
                   a_d=np.asarray(inputs['adf'], np.float32).reshape(HID),
                   we=we_fold(inputs['Wef'], inputs['aef']),
                   b=np.asarray(inputs['bf'], np.float32),
                   Wn=None))

    a_s_all = np.concatenate([rep_row(L['a_s']) for L in Ls], axis=1).astype(BF16)
    a_d_all = np.concatenate([rep_row(L['a_d']) for L in Ls], axis=1).astype(BF16)
    b_all = np.concatenate([rep_row(L['b']) for L in Ls], axis=1).astype(np.float32)
    we_all = np.concatenate([np.tile(L['we'].reshape(1, H), (128, 1)) for L in Ls],
                            axis=1).astype(np.float32)
    Wn_all = np.concatenate([Ls[i]['Wn'] for i in range(3)], axis=1).astype(BF16)

    batch = np.asarray(inputs['batch'], np.int64)
    cnt = np.zeros(NG, np.float64)
    np.add.at(cnt, batch, 1.0)
    pool_mat = np.zeros((NCORES, 128, BPC * NG), np.float32)
    bv = batch[node[valid]]
    pool_mat[k_[valid], p_[valid], b_[valid] * NG + bv] = 1.0

    C2w = np.asarray(inputs['C2w'], np.float32)
    host = dict(
        TLO=TLO, THI=THI, chunks=chunks, ntiles=ntiles,
        n_lo_tiles=n_lo_tiles, n_hi_tiles=n_hi_tiles,
        idx_lo=idx_lo, idx_hi=idx_hi, idx_dst=idx_dst, dstcol=dstcol.astype(BF16),
        latcol=latcol,
        onehot4T=onehot4T.astype(BF16), req_w_full=req_w_full, us_w_full=us_w_full,
        mask_ge15=mask_ge15, mask_lt15=mask_lt15, perms=perms,
        T0=T0.astype(BF16),
        w16_rep=rep_row(np.tile(W0[F], 1)), w17_rep=rep_row(W0[F + 1]),
        a_s_all=a_s_all, a_d_all=a_d_all, b_all=b_all, we_all=we_all,
        Wn_all=Wn_all, cnt=cnt, pool_mat=pool_mat.astype(BF16),
        C1w=np.asarray(inputs['C1w'], np.float32).astype(BF16),
        C2w=np.concatenate([C2w[0:128], C2w[128:256]], axis=1).astype(BF16),
        C3w=np.ascontiguousarray(np.asarray(inputs['C3w'], np.float32).reshape(2, 128).T).astype(BF16),
        c1b_col=np.ascontiguousarray(np.asarray(inputs['C1b'], np.float32).reshape(2, 128).T),
        c2b_col=np.ascontiguousarray(np.asarray(inputs['C2b'], np.float32).reshape(2, 128).T),
        C3b=float(np.asarray(inputs['C3b'], np.float32)[0]),
        iota_row_bf=np.tile(np.arange(128, dtype=np.float32)[None, :], (128, 1)).astype(BF16),
        ident_bf=np.eye(128, dtype=np.float32).astype(BF16),
        ones_col=np.ones((128, 1), np.float32),
    )
    return host


def _build_program(host):
    import concourse.bacc as bacc
    import concourse.mybir as mybir
    import concourse.tile as tile
    from concourse import library_config
    F32 = mybir.dt.float32
    BF = mybir.dt.bfloat16
    I16 = mybir.dt.int16
    ALU = mybir.AluOpType
    AX = mybir.AxisListType
    ACTF = mybir.ActivationFunctionType

    TLO, THI, chunks = host['TLO'], host['THI'], host['chunks']
    ntiles = host['ntiles']
    nlo, nhi = host['n_lo_tiles'], host['n_hi_tiles']

    nc = bacc.Bacc("TRN2", target_bir_lowering=False, debug=False, num_devices=NCORES)

    ins = {
        'idx_lo': ([128, nlo * 8], I16), 'idx_hi': ([128, nhi * 8], I16),
        'idx_dst': ([128, ntiles * 8], I16),
        'dstcol': ([128, ntiles], BF), 'latcol': ([128, ntiles], F32),
        'a_s_all': ([128, NLAYERS * HID], BF), 'a_d_all': ([128, NLAYERS * HID], BF),
        'b_all': ([128, NLAYERS * HID], F32), 'we_all': ([128, NLAYERS * H], F32),
        'Wn_all': ([128, 3 * HID], BF),
        'req_w': ([128, NBLK], F32), 'us_own': ([128, BPC], F32),
        'mask_ge15': ([128, NBLK], F32), 'mask_lt15': ([128, NBLK], F32),
        'onehot4T': ([4, BPC * BLK], BF), 'T0': ([4, HID], BF),
        'w16_rep': ([128, HID], F32), 'w17_rep': ([128, HID], F32),
        'C1w': ([HID, FC], BF), 'C2w': ([128, 2 * FC], BF), 'C3w': ([128, 2], BF),
        'c1b_col': ([128, 2], F32), 'c2b_col': ([128, 2], F32),
        'pool_mat': ([128, BPC * NG], BF),
        'iota_row_bf': ([128, 128], BF), 'ident_bf': ([128, 128], BF),
        'ones_col': ([128, 1], F32),
    }
    tin = {}
    for name, (shape, dt) in ins.items():
        tin[name] = nc.dram_tensor(name, list(shape), dt, kind="ExternalInput")
    t_part = nc.dram_tensor('partials', [NG, 1], F32, kind="ExternalOutput")

    # static chunk metadata: per chunk -> (lo_tile_start, glo, hi_tile_start, ghi,
    #   per-block slot lists, global tile index base)
    chunk_meta = []
    lo_base = hi_base = gi_base = 0
    for blks in chunks:
        glo = int(sum(TLO[b] for b in blks))
        ghi = int(sum(THI[b] for b in blks))
        T = glo + ghi
        # chunk slot s in [0,T): lo slots first (block-major), then hi
        per_block = []     # (b, [slots], [is_lo flags])
        s = 0
        slots_lo = {}
        for b in blks:
            slots_lo[b] = list(range(s, s + int(TLO[b])))
            s += int(TLO[b])
        slots_hi = {}
        for b in blks:
            slots_hi[b] = list(range(s, s + int(THI[b])))
            s += int(THI[b])
        for b in blks:
            per_block.append((b, slots_lo[b] + slots_hi[b]))
        chunk_meta.append(dict(blks=blks, glo=glo, ghi=ghi, T=T,
                               lo0=lo_base, hi0=hi_base, gi0=gi_base,
                               per_block=per_block))
        lo_base += glo
        hi_base += ghi
        gi_base += T

    with tile.TileContext(nc) as tc:
        with (
            tc.tile_pool(name="const", bufs=1) as constp,
            tc.tile_pool(name="gbuf", bufs=2) as gbufp,
            tc.tile_pool(name="ohp", bufs=2) as ohp,
            tc.tile_pool(name="wsp", bufs=2) as wsp,
            tc.tile_pool(name="work", bufs=2) as workp,
            tc.tile_pool(name="slice", bufs=2) as slicep,
            tc.tile_pool(name="psT", bufs=2, space="PSUM") as psT,
            tc.tile_pool(name="psE", bufs=3, space="PSUM") as psE,
            tc.tile_pool(name="psG", bufs=1, space="PSUM") as psG,
            tc.tile_pool(name="psMM", bufs=2, space="PSUM") as psMM,
            tc.tile_pool(name="dram", bufs=2, space="DRAM") as dramp,
        ):
            nc.gpsimd.load_library(library_config.mlp)
            c = {}
            for name in ins:
                shape, dt = ins[name]
                t = constp.tile(list(shape), dt, tag=name, name=name)
                nc.sync.dma_start(t[:], tin[name].ap())
                c[name] = t

            # ---------------- feat phase: xp1 own slice ----------------
            n = float(N - NL)
            d = workp.tile([128, NBLK], F32, tag="fd", name="fd")
            nc.vector.tensor_tensor(out=d[:], in0=c['req_w'][:], in1=c['mask_ge15'][:], op=ALU.mult)
            col = workp.tile([128, 1], F32, tag="fcol", name="fcol")
            nc.vector.tensor_reduce(out=col[:], in_=d[:], op=ALU.add, axis=AX.X)
            tot = psMM.tile([1, 1], F32, tag="mm", name="ftot")
            nc.tensor.matmul(tot[:], col[:], c['ones_col'][:], start=True, stop=True)
            mean = workp.tile([1, 1], F32, tag="fmean", name="fmean")
            nc.vector.tensor_scalar(out=mean[:], in0=tot[:], scalar1=1.0 / n, scalar2=None, op0=ALU.mult)
            mean_col = workp.tile([128, 1], F32, tag="fmc", name="fmc")
            nc.gpsimd.partition_broadcast(mean_col[:], mean[:])
            nc.vector.tensor_scalar(out=d[:], in0=c['req_w'][:], scalar1=mean_col[:, 0:1], scalar2=None, op0=ALU.subtract)
            nc.vector.tensor_tensor(out=d[:], in0=d[:], in1=c['mask_ge15'][:], op=ALU.mult)
            d2 = workp.tile([128, NBLK], F32, tag="fd2", name="fd2")
            nc.vector.tensor_tensor(out=d2[:], in0=d[:], in1=d[:], op=ALU.mult)
            nc.vector.tensor_reduce(out=col[:], in_=d2[:], op=ALU.add, axis=AX.X)
            tot2 = psMM.tile([1, 1], F32, tag="mm", name="ftot2")
            nc.tensor.matmul(tot2[:], col[:], c['ones_col'][:], start=True, stop=True)
            var = workp.tile([1, 1], F32, tag="fvar", name="fvar")
            nc.vector.tensor_scalar(out=var[:], in0=tot2[:], scalar1=1.0 / (n - 1.0), scalar2=None, op0=ALU.mult)
            std = workp.tile([1, 1], F32, tag="fstd", name="fstd")
            nc.scalar.activation(out=std[:], in_=var[:], func=ACTF.Sqrt)
            nc.vector.tensor_scalar(out=std[:], in0=std[:], scalar1=1e-6, scalar2=None, op0=ALU.add)
            rinv = workp.tile([1, 1], F32, tag="frinv", name="frinv")
            nc.vector.reciprocal(out=rinv[:], in_=std[:])
            rinv_col = workp.tile([128, 1], F32, tag="frc", name="frc")
            nc.gpsimd.partition_broadcast(rinv_col[:], rinv[:])
            rf = workp.tile([128, NBLK], F32, tag="frf", name="frf")
            nc.vector.tensor_scalar(out=rf[:], in0=d[:], scalar1=rinv_col[:, 0:1], scalar2=None, op0=ALU.mult)
            raw15 = workp.tile([128, NBLK], F32, tag="fr15", name="fr15")
            nc.vector.tensor_tensor(out=raw15[:], in0=c['req_w'][:], in1=c['mask_lt15'][:], op=ALU.mult)
            nc.vector.tensor_tensor(out=rf[:], in0=rf[:], in1=raw15[:], op=ALU.add)

            xpown = slicep.tile([128, BPC, HID], BF, tag="xpown", name="xpown0")
            for b in range(BPC):
                mm = psMM.tile([128, HID], F32, tag="mm", name="fmm")
                nc.tensor.matmul(mm[:], c['onehot4T'][:, b * 128:(b + 1) * 128], c['T0'][:],
                                 start=True, stop=True)
                x0 = workp.tile([128, HID], F32, tag="fx0", name="fx0")
                t1 = workp.tile([128, HID], F32, tag="ft1", name="ft1")
                nc.vector.tensor_scalar(out=t1[:], in0=c['w16_rep'][:], scalar1=rf[:, b:b + 1], scalar2=None, op0=ALU.mult)
                nc.vector.tensor_tensor(out=x0[:], in0=mm[:], in1=t1[:], op=ALU.add)
                nc.vector.tensor_scalar(out=t1[:], in0=c['w17_rep'][:], scalar1=c['us_own'][:, b:b + 1], scalar2=None, op0=ALU.mult)
                nc.vector.tensor_tensor(out=x0[:], in0=x0[:], in1=t1[:], op=ALU.add)
                nc.scalar.copy(out=xpown[:, b, :], in_=x0[:])

            # ---------------- 4 GAT layers ----------------
            for li in range(NLAYERS):
                a_s = c['a_s_all'][:, li * HID:(li + 1) * HID]
                a_d = c['a_d_all'][:, li * HID:(li + 1) * HID]
                b_rep = c['b_all'][:, li * HID:(li + 1) * HID]
                we_rep = c['we_all'][:, li * H:(li + 1) * H]
                relu = li < NLAYERS - 1

                # ship own xp slice, AllGather full table
                xsl = dramp.tile([BPC * BLK, HID], BF, tag="xsl", name=f"xsl{li}")
                nc.sync.dma_start(xsl[:].rearrange("(b p) j -> p b j", p=128), xpown[:])
                tab = dramp.tile([NPAD, HID], BF, tag="tab", addr_space="Shared", name=f"tab{li}")
                nc.gpsimd.collective_compute(
                    "AllGather", ALU.bypass,
                    replica_groups=[list(range(NCORES))],
                    ins=[xsl[:]], outs=[tab[:]])

                xslice = slicep.tile([128, BPC, HID], BF, tag="xslice", name=f"xslice{li}", bufs=1)

                for cm in chunk_meta:
                    glo, ghi, T = cm['glo'], cm['ghi'], cm['T']
                    g_lo = gbufp.tile([128, max(glo, 1), HID], BF, tag="g_lo", name=f"glo{li}_{cm['gi0']}")
                    g_hi = gbufp.tile([128, max(ghi, 1), HID], BF, tag="g_hi", name=f"ghi{li}_{cm['gi0']}")
                    if glo:
                        nc.gpsimd.dma_gather(
                            g_lo[:, 0:glo, :], tab[0:HALF, :],
                            c['idx_lo'][:, cm['lo0'] * 8:(cm['lo0'] + glo) * 8],
                            glo * 128, glo * 128, HID, single_packet=False)
                    if ghi:
                        nc.gpsimd.dma_gather(
                            g_hi[:, 0:ghi, :], tab[HALF:NPAD, :],
                            c['idx_hi'][:, cm['hi0'] * 8:(cm['hi0'] + ghi) * 8],
                            ghi * 128, ghi * 128, HID, single_packet=False)
                    # dst rows from the core-local slice (no AG dependency)
                    g_dst = gbufp.tile([128, T, HID], BF, tag="g_dst", name=f"gd{li}_{cm['gi0']}")
                    nc.gpsimd.dma_gather(
                        g_dst[:], xsl[:, :],
                        c['idx_dst'][:, cm['gi0'] * 8:(cm['gi0'] + T) * 8],
                        T * 128, T * 128, HID, single_packet=False)

                    # s_src for all chunk slots (lo block-major, then hi)
                    s_src = workp.tile([128, T, H], F32, tag="s_src", name=f"ss{li}_{cm['gi0']}")
                    xa = workp.tile([128, T, HID], BF, tag="xa", name=f"xa{li}_{cm['gi0']}", bufs=1)
                    if glo:
                        nc.vector.tensor_tensor(
                            out=xa[:, 0:glo, :], in0=g_lo[:, 0:glo, :],
                            in1=a_s.rearrange("p f -> p () f").broadcast_to([128, glo, HID]),
                            op=ALU.mult)
                    if ghi:
                        nc.vector.tensor_tensor(
                            out=xa[:, glo:T, :], in0=g_hi[:, 0:ghi, :],
                            in1=a_s.rearrange("p f -> p () f").broadcast_to([128, ghi, HID]),
                            op=ALU.mult)
                    nc.vector.tensor_reduce(
                        out=s_src[:], in_=xa[:].rearrange("p t (h c) -> p t h c", h=H),
                        op=ALU.add, axis=AX.X)

                    # one-hot dst matrices for the chunk
                    oh_ch = ohp.tile([128, T, 128], BF, tag="oh_ch", name=f"oh{li}_{cm['gi0']}")
                    nc.vector.tensor_tensor(
                        out=oh_ch[:],
                        in0=c['iota_row_bf'][:].rearrange("p f -> p () f").broadcast_to([128, T, 128]),
                        in1=c['dstcol'][:, cm['gi0']:cm['gi0'] + T].rearrange("p t -> p t ()").broadcast_to([128, T, 128]),
                        op=ALU.is_equal)

                    # s_dst per edge from gathered dst rows
                    s_dst = workp.tile([128, T, H], F32, tag="s_dst", name=f"sd{li}_{cm['gi0']}")
                    xad = workp.tile([128, T, HID], BF, tag="xa", name=f"xad{li}_{cm['gi0']}", bufs=1)
                    nc.vector.tensor_tensor(
                        out=xad[:], in0=g_dst[:],
                        in1=a_d.rearrange("p f -> p () f").broadcast_to([128, T, HID]),
                        op=ALU.mult)
                    nc.vector.tensor_reduce(
                        out=s_dst[:], in_=xad[:].rearrange("p t (h c) -> p t h c", h=H),
                        op=ALU.add, axis=AX.X)

                    # alpha: s_src + s_dst + we*lat; leaky_relu; exp
                    latw = workp.tile([128, T, H], F32, tag="latw", name=f"lw{li}_{cm['gi0']}")
                    nc.vector.tensor_tensor(
                        out=latw[:],
                        in0=c['latcol'][:, cm['gi0']:cm['gi0'] + T].rearrange("p t -> p t ()").broadcast_to([128, T, H]),
                        in1=we_rep.rearrange("p h -> p () h").broadcast_to([128, T, H]),
                        op=ALU.mult)
                    araw = workp.tile([128, T, H], F32, tag="araw", name=f"ar{li}_{cm['gi0']}")
                    nc.vector.tensor_tensor(out=araw[:], in0=s_src[:], in1=s_dst[:], op=ALU.add)
                    nc.vector.tensor_tensor(out=araw[:], in0=araw[:], in1=latw[:], op=ALU.add)
                    lr = workp.tile([128, T, H], F32, tag="lr", name=f"lr{li}_{cm['gi0']}")
                    nc.vector.tensor_scalar(out=lr[:], in0=araw[:], scalar1=0.2, scalar2=None, op0=ALU.mult)
                    nc.vector.tensor_tensor(out=araw[:], in0=araw[:], in1=lr[:], op=ALU.max)

                    wstack = wsp.tile([128, T, H + HID], BF, tag="wstack", name=f"wst{li}_{cm['gi0']}")
                    wexp = workp.tile([128, T, H], BF, tag="wexp", name=f"we{li}_{cm['gi0']}")
                    nc.scalar.activation(out=wexp[:], in_=araw[:], func=ACTF.Exp)
                    nc.scalar.activation(out=wstack[:, :, 0:H], in_=araw[:], func=ACTF.Exp)
                    # weighted messages from raw gathered features
                    if glo:
                        nc.vector.tensor_tensor(
                            out=wstack[:, 0:glo, H:].rearrange("p t (h c) -> p t h c", h=H),
                            in0=g_lo[:, 0:glo, :].rearrange("p t (h c) -> p t h c", h=H),
                            in1=wexp[:, 0:glo, :].rearrange("p t h -> p t h ()").broadcast_to([128, glo, H, C]),
                            op=ALU.mult)
                    if ghi:
                        nc.vector.tensor_tensor(
                            out=wstack[:, glo:T, H:].rearrange("p t (h c) -> p t h c", h=H),
                            in0=g_hi[:, 0:ghi, :].rearrange("p t (h c) -> p t h c", h=H),
                            in1=wexp[:, glo:T, :].rearrange("p t h -> p t h ()").broadcast_to([128, ghi, H, C]),
                            op=ALU.mult)

                    # scatter per dst block: [den | num] accumulated on PE
                    for b, slots in cm['per_block']:
                        acc = psE.tile([128, H + HID], F32, tag="eacc", name=f"acc{li}_{b}")
                        for j, t in enumerate(slots):
                            nc.tensor.matmul(acc[:], oh_ch[:, t, :], wstack[:, t, :],
                                             start=(j == 0), stop=(j == len(slots) - 1))
                        den = workp.tile([128, H], F32, tag="den", name=f"den{li}_{b}")
                        nc.vector.tensor_scalar(out=den[:], in0=acc[:, 0:H], scalar1=1e-16, scalar2=None, op0=ALU.add)
                        recip = workp.tile([128, H], F32, tag="recip", name=f"rc{li}_{b}")
                        nc.vector.reciprocal(out=recip[:], in_=den[:])
                        xn = workp.tile([128, HID], F32, tag="xn", name=f"xn{li}_{b}")
                        nc.vector.tensor_tensor(
                            out=xn[:].rearrange("p (h c) -> p h c", h=H),
                            in0=acc[:, H:].rearrange("p (h c) -> p h c", h=H),
                            in1=recip[:].rearrange("p h -> p h ()").broadcast_to([128, H, C]),
                            op=ALU.mult)
                        nc.vector.tensor_tensor(out=xn[:], in0=xn[:], in1=b_rep, op=ALU.add)
                        if relu:
                            nc.scalar.activation(out=xslice[:, b, :], in_=xn[:], func=ACTF.Relu)
                        else:
                            nc.scalar.copy(out=xslice[:, b, :], in_=xn[:])

                if li < NLAYERS - 1:
                    Wn = c['Wn_all'][:, li * HID:(li + 1) * HID]
                    xpown = slicep.tile([128, BPC, HID], BF, tag="xpown", name=f"xpown{li + 1}")
                    for b in range(BPC):
                        tp = psT.tile([128, 128], BF, tag="tp", name=f"ntp{li}_{b}")
                        nc.tensor.transpose(tp[:], xslice[:, b, :], c['ident_bf'][:])
                        xT = workp.tile([128, 128], BF, tag="xT", name=f"nxT{li}_{b}")
                        nc.scalar.copy(out=xT[:], in_=tp[:])
                        xpp = psMM.tile([128, HID], F32, tag="mm", name=f"nxpp{li}_{b}")
                        nc.tensor.matmul(xpp[:], xT[:], Wn, start=True, stop=True)
                        nc.scalar.copy(out=xpown[:, b, :], in_=xpp[:])

            # ---------------- MLP head + pool ----------------
            gp = psG.tile([NG, 1], F32, tag="gp", name="gp")
            for b in range(BPC):
                tp = psT.tile([128, 128], BF, tag="tp", name=f"mtp{b}")
                nc.tensor.transpose(tp[:], xslice[:, b, :], c['ident_bf'][:])
                xT = workp.tile([128, 128], BF, tag="xT", name=f"mxT{b}")
                nc.scalar.copy(out=xT[:], in_=tp[:])
                h1 = []
                for jh in range(2):
                    hp = psMM.tile([128, 128], F32, tag="mm", name=f"mh1p{b}_{jh}")
                    nc.tensor.matmul(hp[:], c['C1w'][:, jh * 128:(jh + 1) * 128], xT[:],
                                     start=True, stop=True)
                    hs = workp.tile([128, 128], BF, tag=f"h1_{jh}", name=f"mh1s{b}_{jh}")
                    nc.vector.tensor_scalar(out=hs[:], in0=hp[:],
                                            scalar1=c['c1b_col'][:, jh:jh + 1],
                                            scalar2=0.0, op0=ALU.add, op1=ALU.max)
                    h1.append(hs)
                h2 = []
                for jh in range(2):
                    hp = psMM.tile([128, 128], F32, tag="mm", name=f"mh2p{b}_{jh}")
                    for kc in range(2):
                        nc.tensor.matmul(hp[:], c['C2w'][:, kc * FC + jh * 128:kc * FC + (jh + 1) * 128],
                                         h1[kc][:], start=(kc == 0), stop=(kc == 1))
                    hs = workp.tile([128, 128], BF, tag=f"h2_{jh}", name=f"mh2s{b}_{jh}")
                    nc.vector.tensor_scalar(out=hs[:], in0=hp[:],
                                            scalar1=c['c2b_col'][:, jh:jh + 1],
                                            scalar2=0.0, op0=ALU.add, op1=ALU.max)
                    h2.append(hs)
                nvp = psMM.tile([128, 1], F32, tag="mm", name=f"mnvp{b}")
                for kc in range(2):
                    nc.tensor.matmul(nvp[:], h2[kc][:], c['C3w'][:, kc:kc + 1],
                                     start=(kc == 0), stop=(kc == 1))
                nv = workp.tile([128, 1], BF, tag="nv", name=f"mnv{b}")
                nc.vector.tensor_scalar(out=nv[:], in0=nvp[:], scalar1=host['C3b'],
                                        scalar2=0.0, op0=ALU.add, op1=ALU.max)
                nc.tensor.matmul(gp[:], c['pool_mat'][:, b * NG:(b + 1) * NG], nv[:],
                                 start=(b == 0), stop=(b == BPC - 1))
            pt = workp.tile([NG, 1], F32, tag="pt", name="pt")
            nc.scalar.copy(out=pt[:], in_=gp[:])
            nc.sync.dma_start(t_part.ap(), pt[:])

    nc.compile()
    return nc


def kernel(**inputs):
    from concourse.bass_utils import run_bass_kernel_spmd
    import hashlib
    inputs = {k: np.asarray(v) for k, v in inputs.items()}
    host = _build_host(inputs)
    key = hashlib.sha1(np.ascontiguousarray(inputs['edge_index']).tobytes()).hexdigest()
    if key not in _cache:
        _cache[key] = _build_program(host)
    prog = _cache[key]

    in_maps = []
    for k in range(NCORES):
        perm = host['perms'][k]
        in_maps.append(dict(
            idx_lo=host['idx_lo'][k], idx_hi=host['idx_hi'][k],
            idx_dst=host['idx_dst'][k],
            dstcol=host['dstcol'][k], latcol=host['latcol'][k],
            a_s_all=host['a_s_all'], a_d_all=host['a_d_all'],
            b_all=host['b_all'], we_all=host['we_all'], Wn_all=host['Wn_all'],
            req_w=np.ascontiguousarray(host['req_w_full'][:, perm]),
            us_own=np.ascontiguousarray(host['us_w_full'][:, k * BPC:(k + 1) * BPC]),
            mask_ge15=np.ascontiguousarray(host['mask_ge15'][:, perm]),
            mask_lt15=np.ascontiguousarray(host['mask_lt15'][:, perm]),
            onehot4T=host['onehot4T'][k], T0=host['T0'],
            w16_rep=host['w16_rep'], w17_rep=host['w17_rep'],
            C1w=host['C1w'], C2w=host['C2w'], C3w=host['C3w'],
            c1b_col=host['c1b_col'], c2b_col=host['c2b_col'],
            pool_mat=host['pool_mat'][k],
            iota_row_bf=host['iota_row_bf'], ident_bf=host['ident_bf'],
            ones_col=host['ones_col'],
        ))
    def _launch():
        t0 = time.monotonic()
        res = run_bass_kernel_spmd(prog, in_maps, core_ids=list(range(NCORES)))
        wall = (time.monotonic() - t0) * 1e9
        t = res.exec_time_ns if res.exec_time_ns else wall
        p = sum(np.asarray(res.results[k]['partials'], np.float64) for k in range(NCORES))
        return p, t

    # The axon terminal occasionally returns corrupted results right after a
    # device reset; run twice (second launch is cheap in-process) and verify.
    times = []
    p1, t = _launch()
    times.append(t)
    p2, t = _launch()
    times.append(t)
    if not np.allclose(p1, p2, rtol=1e-3, atol=1e-6):
        p3, t = _launch()
        times.append(t)
        if np.allclose(p2, p3, rtol=1e-3, atol=1e-6):
            p1 = p2
        elif np.allclose(p1, p3, rtol=1e-3, atol=1e-6):
            pass
        else:
            p1 = p3
    partials = p1
    out = (partials[:, 0] / np.maximum(host['cnt'], 1.0)).astype(np.float32)[:, None]
    kernel._last_times = times
    return out


# revision 5
# speedup vs baseline: 3.4927x; 2.6152x over previous
"""CriticSwapGNN Trainium2 fused kernel: feat + 4 GAT layers + MLP head + pool
in ONE launch across 8 cores, with on-device AllGather of xp between layers.

Sharding: dst-range ownership, 8 cores x 49 blocks of 128 nodes. Edges sorted
by dst block, split lo/hi by src half (int16 gather indices), tiled 128/tile,
grouped in chunks of CHUNK_BLKS dst blocks. Per chunk: dma_gather of src rows
from the AllGathered table plus dst rows from the core-local slice (s_src and
s_dst both computed per-edge on DVE), segment softmax without max-subtraction
(logits are bounded), one fused [den|num] scatter matmul per tile into PSUM.
Table and matmul operands in bf16 (rel err ~4.5e-3 vs 2e-2 gate); accumulations
in fp32 PSUM. The launch is run twice and cross-checked (the axon terminal
occasionally returns corrupted results right after a device reset).
"""
import sys
import time
import numpy as np
import ml_dtypes

if '/opt/trn_rl_repo' not in sys.path:
    sys.path.insert(0, '/opt/trn_rl_repo')

N = 50000; E = 800000; F = 16; HID = 128; H = 4; C = 32; FC = 256; NL = 15; NG = 8
NCORES = 8
BLK = 128
BPC = 49                      # blocks per core
NBLK = NCORES * BPC           # 392
NPAD = NBLK * BLK             # 50176
HALF = 4 * BPC * BLK          # 25088
CHUNK_BLKS = 2
NLAYERS = 4

_cache = {}
BF16 = ml_dtypes.bfloat16


def _chunk_layout():
    chunks = []
    b = 0
    while b < BPC:
        chunks.append(list(range(b, min(b + CHUNK_BLKS, BPC))))
        b += CHUNK_BLKS
    return chunks


def _build_host(inputs):
    src = np.asarray(inputs['edge_index'][0], np.int64)
    dst = np.asarray(inputs['edge_index'][1], np.int64)
    lat = np.asarray(inputs['latency'], np.float32)

    order = np.argsort(dst, kind='stable')
    es, ed, el = src[order], dst[order], lat[order]
    blk_of = ed // BLK
    blk_starts = np.searchsorted(blk_of, np.arange(NBLK + 1))

    # per (core, block): lo/hi edge lists
    per = {}
    nlo = np.zeros((NCORES, BPC), np.int64)
    nhi = np.zeros((NCORES, BPC), np.int64)
    for k in range(NCORES):
        for b in range(BPC):
            g = k * BPC + b
            s_, e_ = blk_starts[g], blk_starts[g + 1]
            bs, bd, bl = es[s_:e_], ed[s_:e_] - g * BLK, el[s_:e_]
            lo = bs < HALF
            per[(k, b)] = (bs[lo], bd[lo], bl[lo], bs[~lo] - HALF, bd[~lo], bl[~lo])
            nlo[k, b] = len(bs[lo])
            nhi[k, b] = len(bs) - nlo[k, b]
    TLO = -(-nlo.max(axis=0) // 128)      # uniform tiles per block (lo half)
    THI = -(-nhi.max(axis=0) // 128)

    chunks = _chunk_layout()
    # global tile axis: chunk-major; within chunk: lo tiles (block-major), hi tiles
    tile_axis = []
    for blks in chunks:
        for b in blks:
            for t in range(int(TLO[b])):
                tile_axis.append(('lo', b, t))
        for b in blks:
            for t in range(int(THI[b])):
                tile_axis.append(('hi', b, t))
    ntiles = len(tile_axis)
    gi_of = {v: i for i, v in enumerate(tile_axis)}
    lo_order = [v for v in tile_axis if v[0] == 'lo']   # gather order, chunk-major
    hi_order = [v for v in tile_axis if v[0] == 'hi']
    lo_pos = {v: i for i, v in enumerate(lo_order)}
    hi_pos = {v: i for i, v in enumerate(hi_order)}
    n_lo_tiles, n_hi_tiles = len(lo_order), len(hi_order)

    idx_lo = np.zeros((NCORES, 128, n_lo_tiles * 8), np.int16)
    idx_hi = np.zeros((NCORES, 128, n_hi_tiles * 8), np.int16)
    idx_dst = np.zeros((NCORES, 128, ntiles * 8), np.int16)
    dstcol = np.full((NCORES, 128, ntiles), float(BLK), np.float32)
    latcol = np.zeros((NCORES, 128, ntiles), np.float32)

    def wrap16(a):     # [128] int -> [128, 8] int16 wrapped+replicated
        return np.tile(a.astype(np.int16).reshape(-1, 16).T, (8, 1))

    for k in range(NCORES):
        for b in range(BPC):
            slo, dlo, llo, shi, dhi, lhi = per[(k, b)]
            for half, s_, d_, l_, T_, pos, idx_arr in (
                    ('lo', slo, dlo, llo, TLO, lo_pos, idx_lo),
                    ('hi', shi, dhi, lhi, THI, hi_pos, idx_hi)):
                nt = int(T_[b])
                if nt == 0:
                    continue
                cap = nt * 128
                sp = np.zeros(cap, np.int64)
                dp = np.full(cap, BLK, np.int64)
                dl = np.zeros(cap, np.int64)        # dst local node (pad -> 0)
                lp = np.zeros(cap, np.float32)
                sp[:len(s_)] = s_
                dp[:len(d_)] = d_
                dl[:len(d_)] = b * BLK + d_
                lp[:len(l_)] = l_
                for t in range(nt):
                    p = pos[(half, b, t)]
                    idx_arr[k][:, p * 8:(p + 1) * 8] = wrap16(sp[t * 128:(t + 1) * 128])
                    gi = gi_of[(half, b, t)]
                    idx_dst[k][:, gi * 8:(gi + 1) * 8] = wrap16(dl[t * 128:(t + 1) * 128])
                    dstcol[k][:, gi] = dp[t * 128:(t + 1) * 128]
                    latcol[k][:, gi] = lp[t * 128:(t + 1) * 128]

    # ---- features ----
    type_ids = np.asarray(inputs['type_ids'], np.int64)
    node = np.arange(NPAD)
    valid = node < N
    k_ = node // (BPC * BLK)
    b_ = (node % (BPC * BLK)) // BLK
    p_ = node % BLK
    onehot4T = np.zeros((NCORES, 4, BPC * BLK), np.float32)
    tid = np.full(NPAD, -1, np.int64)
    tid[:N] = type_ids
    for t in range(4):
        m = tid == t
        onehot4T[k_[m], t, b_[m] * BLK + p_[m]] = 1.0

    def wrapnode(x):   # [N] -> [128, NBLK]
        o = np.zeros(NPAD, np.float32)
        o[:N] = x
        return o.reshape(-1, 128).T.copy()

    req_w_full = wrapnode(np.asarray(inputs['requests'], np.float32))
    us_w_full = wrapnode(np.asarray(inputs['update_step'], np.float32))
    idx_node = np.arange(NPAD).reshape(-1, 128).T
    mask_ge15 = ((idx_node >= NL) & (idx_node < N)).astype(np.float32)
    mask_lt15 = (idx_node < NL).astype(np.float32)

    perms = []
    for k in range(NCORES):
        own = np.arange(k * BPC, (k + 1) * BPC)
        rest = np.array([c for c in range(NBLK) if not (k * BPC <= c < (k + 1) * BPC)])
        perms.append(np.concatenate([own, rest]))

    def rep_row(v):    # [HID] -> [128, HID]
        return np.tile(np.asarray(v, np.float32).reshape(1, -1), (128, 1))

    def we_fold(We, a_e):
        We = np.asarray(We, np.float32).reshape(1, HID)
        a_e = np.asarray(a_e, np.float32)
        return np.array([(We[0, h * C:(h + 1) * C] * a_e[h]).sum() for h in range(H)],
                        np.float32)

    W0 = np.asarray(inputs['W0'], np.float32)
    T0 = (np.asarray(inputs['emb'], np.float32) @ W0[:F]).astype(np.float32)

    Ls = []
    Ls.append(dict(a_s=np.asarray(inputs['as0'], np.float32).reshape(HID),
                   a_d=np.asarray(inputs['ad0'], np.float32).reshape(HID),
                   we=we_fold(inputs['We0'], inputs['ae0']),
                   b=np.asarray(inputs['b0'], np.float32),
                   Wn=np.asarray(inputs['Wh'][0], np.float32)))
    Ls.append(dict(a_s=np.asarray(inputs['ash'][0], np.float32).reshape(HID),
                   a_d=np.asarray(inputs['adh'][0], np.float32).reshape(HID),
                   we=we_fold(np.asarray(inputs['Weh'][0]).reshape(1, -1), inputs['aeh'][0]),
                   b=np.asarray(inputs['bh'][0], np.float32),
                   Wn=np.asarray(inputs['Wh'][1], np.float32)))
    Ls.append(dict(a_s=np.asarray(inputs['ash'][1], np.float32).reshape(HID),
                   a_d=np.asarray(inputs['adh'][1], np.float32).reshape(HID),
                   we=we_fold(np.asarray(inputs['Weh'][1]).reshape(1, -1), inputs['aeh'][1]),
                   b=np.asarray(inputs['bh'][1], np.float32),
                   Wn=np.asarray(inputs['Wf'], np.float32)))
    Ls.append(dict(a_s=np.asarray(inputs['asf'], np.float32).reshape(HID),
                   a_d=np.asarray(inputs['adf'], np.float32).reshape(HID),
                   we=we_fold(inputs['Wef'], inputs['aef']),
                   b=np.asarray(inputs['bf'], np.float32),
                   Wn=None))

    a_s_all = np.concatenate([rep_row(L['a_s']) for L in Ls], axis=1).astype(BF16)
    a_d_all = np.concatenate([rep_row(L['a_d']) for L in Ls], axis=1).astype(BF16)
    b_all = np.concatenate([rep_row(L['b']) for L in Ls], axis=1).astype(np.float32)
    we_all = np.concatenate([np.tile(L['we'].reshape(1, H), (128, 1)) for L in Ls],
                            axis=1).astype(np.float32)
    Wn_all = np.concatenate([Ls[i]['Wn'] for i in range(3)], axis=1).astype(BF16)

    batch = np.asarray(inputs['batch'], np.int64)
    cnt = np.zeros(NG, np.float64)
    np.add.at(cnt, batch, 1.0)
    pool_mat = np.zeros((NCORES, 128, BPC * NG), np.float32)
    bv = batch[node[valid]]
    pool_mat[k_[valid], p_[valid], b_[valid] * NG + bv] = 1.0

    C2w = np.asarray(inputs['C2w'], np.float32)
    host = dict(
        TLO=TLO, THI=THI, chunks=chunks, ntiles=ntiles,
        n_lo_tiles=n_lo_tiles, n_hi_tiles=n_hi_tiles,
        idx_lo=idx_lo, idx_hi=idx_hi, idx_dst=idx_dst, dstcol=dstcol.astype(BF16),
        latcol=latcol,
        onehot4T=onehot4T.astype(BF16), req_w_full=req_w_full, us_w_full=us_w_full,
        mask_ge15=mask_ge15, mask_lt15=mask_lt15, perms=perms,
        T0=T0.astype(BF16),
        w16_rep=rep_row(np.tile(W0[F], 1)), w17_rep=rep_row(W0[F + 1]),
        a_s_all=a_s_all, a_d_all=a_d_all, b_all=b_all, we_all=we_all,
        Wn_all=Wn_all, cnt=cnt, pool_mat=pool_mat.astype(BF16),
        C1w=np.asarray(inputs['C1w'], np.float32).astype(BF16),
        C2w=np.concatenate([C2w[0:128], C2w[128:256]], axis=1).astype(BF16),
        C3w=np.ascontiguousarray(np.asarray(inputs['C3w'], np.float32).reshape(2, 128).T).astype(BF16),
        c1b_col=np.ascontiguousarray(np.asarray(inputs['C1b'], np.float32).reshape(2, 128).T),
        c2b_col=np.ascontiguousarray(np.asarray(inputs['C2b'], np.float32).reshape(2, 128).T),
        C3b=float(np.asarray(inputs['C3b'], np.float32)[0]),
        iota_row_bf=np.tile(np.arange(128, dtype=np.float32)[None, :], (128, 1)).astype(BF16),
        ident_bf=np.eye(128, dtype=np.float32).astype(BF16),
        ones_col=np.ones((128, 1), np.float32),
    )
    return host


def _build_program(host):
    import concourse.bacc as bacc
    import concourse.mybir as mybir
    import concourse.tile as tile
    from concourse import library_config
    F32 = mybir.dt.float32
    BF = mybir.dt.bfloat16
    I16 = mybir.dt.int16
    ALU = mybir.AluOpType
    AX = mybir.AxisListType
    ACTF = mybir.ActivationFunctionType

    TLO, THI, chunks = host['TLO'], host['THI'], host['chunks']
    ntiles = host['ntiles']
    nlo, nhi = host['n_lo_tiles'], host['n_hi_tiles']

    nc = bacc.Bacc("TRN2", target_bir_lowering=False, debug=False, num_devices=NCORES)

    ins = {
        'idx_lo': ([128, nlo * 8], I16), 'idx_hi': ([128, nhi * 8], I16),
        'idx_dst': ([128, ntiles * 8], I16),
        'dstcol': ([128, ntiles], BF), 'latcol': ([128, ntiles], F32),
        'a_s_all': ([128, NLAYERS * HID], BF), 'a_d_all': ([128, NLAYERS * HID], BF),
        'b_all': ([128, NLAYERS * HID], F32), 'we_all': ([128, NLAYERS * H], F32),
        'Wn_all': ([128, 3 * HID], BF),
        'req_w': ([128, NBLK], F32), 'us_own': ([128, BPC], F32),
        'mask_ge15': ([128, NBLK], F32), 'mask_lt15': ([128, NBLK], F32),
        'onehot4T': ([4, BPC * BLK], BF), 'T0': ([4, HID], BF),
        'w16_rep': ([128, HID], F32), 'w17_rep': ([128, HID], F32),
        'C1w': ([HID, FC], BF), 'C2w': ([128, 2 * FC], BF), 'C3w': ([128, 2], BF),
        'c1b_col': ([128, 2], F32), 'c2b_col': ([128, 2], F32),
        'pool_mat': ([128, BPC * NG], BF),
        'iota_row_bf': ([128, 128], BF), 'ident_bf': ([128, 128], BF),
        'ones_col': ([128, 1], F32),
    }
    tin = {}
    for name, (shape, dt) in ins.items():
        tin[name] = nc.dram_tensor(name, list(shape), dt, kind="ExternalInput")
    t_part = nc.dram_tensor('partials', [NG, 1], F32, kind="ExternalOutput")

    # static chunk metadata: per chunk -> (lo_tile_start, glo, hi_tile_start, ghi,
    #   per-block slot lists, global tile index base)
    chunk_meta = []
    lo_base = hi_base = gi_base = 0
    for blks in chunks:
        glo = int(sum(TLO[b] for b in blks))
        ghi = int(sum(THI[b] for b in blks))
        T = glo + ghi
        # chunk slot s in [0,T): lo slots first (block-major), then hi
        per_block = []     # (b, [slots], [is_lo flags])
        s = 0
        slots_lo = {}
        for b in blks:
            slots_lo[b] = list(range(s, s + int(TLO[b])))
            s += int(TLO[b])
        slots_hi = {}
        for b in blks:
            slots_hi[b] = list(range(s, s + int(THI[b])))
            s += int(THI[b])
        for b in blks:
            per_block.append((b, slots_lo[b] + slots_hi[b]))
        chunk_meta.append(dict(blks=blks, glo=glo, ghi=ghi, T=T,
                               lo0=lo_base, hi0=hi_base, gi0=gi_base,
                               per_block=per_block))
        lo_base += glo
        hi_base += ghi
        gi_base += T

    with tile.TileContext(nc) as tc:
        with (
            tc.tile_pool(name="const", bufs=1) as constp,
            tc.tile_pool(name="gbuf", bufs=2) as gbufp,
            tc.tile_pool(name="ohp", bufs=2) as ohp,
            tc.tile_pool(name="wsp", bufs=2) as wsp,
            tc.tile_pool(name="work", bufs=2) as workp,
            tc.tile_pool(name="slice", bufs=2) as slicep,
            tc.tile_pool(name="psT", bufs=2, space="PSUM") as psT,
            tc.tile_pool(name="psE", bufs=3, space="PSUM") as psE,
            tc.tile_pool(name="psG", bufs=1, space="PSUM") as psG,
            tc.tile_pool(name="psMM", bufs=2, space="PSUM") as psMM,
            tc.tile_pool(name="dram", bufs=2, space="DRAM") as dramp,
        ):
            nc.gpsimd.load_library(library_config.mlp)
            c = {}
            for name in ins:
                shape, dt = ins[name]
                t = constp.tile(list(shape), dt, tag=name, name=name)
                nc.sync.dma_start(t[:], tin[name].ap())
                c[name] = t

            # ---------------- feat phase: xp1 own slice ----------------
            n = float(N - NL)
            d = workp.tile([128, NBLK], F32, tag="fd", name="fd")
            nc.vector.tensor_tensor(out=d[:], in0=c['req_w'][:], in1=c['mask_ge15'][:], op=ALU.mult)
            col = workp.tile([128, 1], F32, tag="fcol", name="fcol")
            nc.vector.tensor_reduce(out=col[:], in_=d[:], op=ALU.add, axis=AX.X)
            tot = psMM.tile([1, 1], F32, tag="mm", name="ftot")
            nc.tensor.matmul(tot[:], col[:], c['ones_col'][:], start=True, stop=True)
            mean = workp.tile([1, 1], F32, tag="fmean", name="fmean")
            nc.vector.tensor_scalar(out=mean[:], in0=tot[:], scalar1=1.0 / n, scalar2=None, op0=ALU.mult)
            mean_col = workp.tile([128, 1], F32, tag="fmc", name="fmc")
            nc.gpsimd.partition_broadcast(mean_col[:], mean[:])
            nc.vector.tensor_scalar(out=d[:], in0=c['req_w'][:], scalar1=mean_col[:, 0:1], scalar2=None, op0=ALU.subtract)
            nc.vector.tensor_tensor(out=d[:], in0=d[:], in1=c['mask_ge15'][:], op=ALU.mult)
            d2 = workp.tile([128, NBLK], F32, tag="fd2", name="fd2")
            nc.vector.tensor_tensor(out=d2[:], in0=d[:], in1=d[:], op=ALU.mult)
            nc.vector.tensor_reduce(out=col[:], in_=d2[:], op=ALU.add, axis=AX.X)
            tot2 = psMM.tile([1, 1], F32, tag="mm", name="ftot2")
            nc.tensor.matmul(tot2[:], col[:], c['ones_col'][:], start=True, stop=True)
            var = workp.tile([1, 1], F32, tag="fvar", name="fvar")
            nc.vector.tensor_scalar(out=var[:], in0=tot2[:], scalar1=1.0 / (n - 1.0), scalar2=None, op0=ALU.mult)
            std = workp.tile([1, 1], F32, tag="fstd", name="fstd")
            nc.scalar.activation(out=std[:], in_=var[:], func=ACTF.Sqrt)
            nc.vector.tensor_scalar(out=std[:], in0=std[:], scalar1=1e-6, scalar2=None, op0=ALU.add)
            rinv = workp.tile([1, 1], F32, tag="frinv", name="frinv")
            nc.vector.reciprocal(out=rinv[:], in_=std[:])
            rinv_col = workp.tile([128, 1], F32, tag="frc", name="frc")
            nc.gpsimd.partition_broadcast(rinv_col[:], rinv[:])
            rf = workp.tile([128, NBLK], F32, tag="frf", name="frf")
            nc.vector.tensor_scalar(out=rf[:], in0=d[:], scalar1=rinv_col[:, 0:1], scalar2=None, op0=ALU.mult)
            raw15 = workp.tile([128, NBLK], F32, tag="fr15", name="fr15")
            nc.vector.tensor_tensor(out=raw15[:], in0=c['req_w'][:], in1=c['mask_lt15'][:], op=ALU.mult)
            nc.vector.tensor_tensor(out=rf[:], in0=rf[:], in1=raw15[:], op=ALU.add)

            xpown = slicep.tile([128, BPC, HID], BF, tag="xpown", name="xpown0")
            for b in range(BPC):
                mm = psMM.tile([128, HID], F32, tag="mm", name="fmm")
                nc.tensor.matmul(mm[:], c['onehot4T'][:, b * 128:(b + 1) * 128], c['T0'][:],
                                 start=True, stop=True)
                x0 = workp.tile([128, HID], F32, tag="fx0", name="fx0")
                t1 = workp.tile([128, HID], F32, tag="ft1", name="ft1")
                nc.vector.tensor_scalar(out=t1[:], in0=c['w16_rep'][:], scalar1=rf[:, b:b + 1], scalar2=None, op0=ALU.mult)
                nc.vector.tensor_tensor(out=x0[:], in0=mm[:], in1=t1[:], op=ALU.add)
                nc.vector.tensor_scalar(out=t1[:], in0=c['w17_rep'][:], scalar1=c['us_own'][:, b:b + 1], scalar2=None, op0=ALU.mult)
                nc.vector.tensor_tensor(out=x0[:], in0=x0[:], in1=t1[:], op=ALU.add)
                nc.scalar.copy(out=xpown[:, b, :], in_=x0[:])

            # ---------------- 4 GAT layers ----------------
            for li in range(NLAYERS):
                a_s = c['a_s_all'][:, li * HID:(li + 1) * HID]
                a_d = c['a_d_all'][:, li * HID:(li + 1) * HID]
                b_rep = c['b_all'][:, li * HID:(li + 1) * HID]
                we_rep = c['we_all'][:, li * H:(li + 1) * H]
                relu = li < NLAYERS - 1

                # ship own xp slice, AllGather full table
                xsl = dramp.tile([BPC * BLK, HID], BF, tag="xsl", name=f"xsl{li}")
                nc.sync.dma_start(xsl[:].rearrange("(b p) j -> p b j", p=128), xpown[:])
                tab = dramp.tile([NPAD, HID], BF, tag="tab", addr_space="Shared", name=f"tab{li}")
                nc.gpsimd.collective_compute(
                    "AllGather", ALU.bypass,
                    replica_groups=[list(range(NCORES))],
                    ins=[xsl[:]], outs=[tab[:]])

                xslice = slicep.tile([128, BPC, HID], BF, tag="xslice", name=f"xslice{li}", bufs=1)

                for cm in chunk_meta:
                    glo, ghi, T = cm['glo'], cm['ghi'], cm['T']
                    g_lo = gbufp.tile([128, max(glo, 1), HID], BF, tag="g_lo", name=f"glo{li}_{cm['gi0']}")
                    g_hi = gbufp.tile([128, max(ghi, 1), HID], BF, tag="g_hi", name=f"ghi{li}_{cm['gi0']}")
                    if glo:
                        nc.gpsimd.dma_gather(
                            g_lo[:, 0:glo, :], tab[0:HALF, :],
                            c['idx_lo'][:, cm['lo0'] * 8:(cm['lo0'] + glo) * 8],
                            glo * 128, glo * 128, HID, single_packet=False)
                    if ghi:
                        nc.gpsimd.dma_gather(
                            g_hi[:, 0:ghi, :], tab[HALF:NPAD, :],
                            c['idx_hi'][:, cm['hi0'] * 8:(cm['hi0'] + ghi) * 8],
                            ghi * 128, ghi * 128, HID, single_packet=False)
                    # dst rows from the core-local slice (no AG dependency)
                    g_dst = gbufp.tile([128, T, HID], BF, tag="g_dst", name=f"gd{li}_{cm['gi0']}")
                    nc.gpsimd.dma_gather(
                        g_dst[:], xsl[:, :],
                        c['idx_dst'][:, cm['gi0'] * 8:(cm['gi0'] + T) * 8],
                        T * 128, T * 128, HID, single_packet=False)

                    # s_src for all chunk slots (lo block-major, then hi)
                    s_src = workp.tile([128, T, H], F32, tag="s_src", name=f"ss{li}_{cm['gi0']}")
                    xa = workp.tile([128, T, HID], BF, tag="xa", name=f"xa{li}_{cm['gi0']}", bufs=1)
                    if glo:
                        nc.vector.tensor_tensor(
                            out=xa[:, 0:glo, :], in0=g_lo[:, 0:glo, :],
                            in1=a_s.rearrange("p f -> p () f").broadcast_to([128, glo, HID]),
                            op=ALU.mult)
                    if ghi:
                        nc.vector.tensor_tensor(
                            out=xa[:, glo:T, :], in0=g_hi[:, 0:ghi, :],
                            in1=a_s.rearrange("p f -> p () f").broadcast_to([128, ghi, HID]),
                            op=ALU.mult)
                    nc.vector.tensor_reduce(
                        out=s_src[:], in_=xa[:].rearrange("p t (h c) -> p t h c", h=H),
                        op=ALU.add, axis=AX.X)

                    # one-hot dst matrices for the chunk
                    oh_ch = ohp.tile([128, T, 128], BF, tag="oh_ch", name=f"oh{li}_{cm['gi0']}")
                    nc.vector.tensor_tensor(
                        out=oh_ch[:],
                        in0=c['iota_row_bf'][:].rearrange("p f -> p () f").broadcast_to([128, T, 128]),
                        in1=c['dstcol'][:, cm['gi0']:cm['gi0'] + T].rearrange("p t -> p t ()").broadcast_to([128, T, 128]),
                        op=ALU.is_equal)

                    # s_dst per edge from gathered dst rows
                    s_dst = workp.tile([128, T, H], F32, tag="s_dst", name=f"sd{li}_{cm['gi0']}")
                    xad = workp.tile([128, T, HID], BF, tag="xa", name=f"xad{li}_{cm['gi0']}", bufs=1)
                    nc.vector.tensor_tensor(
                        out=xad[:], in0=g_dst[:],
                        in1=a_d.rearrange("p f -> p () f").broadcast_to([128, T, HID]),
                        op=ALU.mult)
                    nc.vector.tensor_reduce(
                        out=s_dst[:], in_=xad[:].rearrange("p t (h c) -> p t h c", h=H),
                        op=ALU.add, axis=AX.X)

                    # alpha: s_src + s_dst + we*lat; leaky_relu; exp
                    latw = workp.tile([128, T, H], F32, tag="latw", name=f"lw{li}_{cm['gi0']}")
                    nc.vector.tensor_tensor(
                        out=latw[:],
                        in0=c['latcol'][:, cm['gi0']:cm['gi0'] + T].rearrange("p t -> p t ()").broadcast_to([128, T, H]),
                        in1=we_rep.rearrange("p h -> p () h").broadcast_to([128, T, H]),
                        op=ALU.mult)
                    araw = workp.tile([128, T, H], F32, tag="araw", name=f"ar{li}_{cm['gi0']}")
                    nc.vector.tensor_tensor(out=araw[:], in0=s_src[:], in1=s_dst[:], op=ALU.add)
                    nc.vector.tensor_tensor(out=araw[:], in0=araw[:], in1=latw[:], op=ALU.add)
                    lr = workp.tile([128, T, H], F32, tag="lr", name=f"lr{li}_{cm['gi0']}")
                    nc.vector.tensor_scalar(out=lr[:], in0=araw[:], scalar1=0.2, scalar2=None, op0=ALU.mult)
                    nc.vector.tensor_tensor(out=araw[:], in0=araw[:], in1=lr[:], op=ALU.max)

                    wstack = wsp.tile([128, T, H + HID], BF, tag="wstack", name=f"wst{li}_{cm['gi0']}")
                    wexp = workp.tile([128, T, H], BF, tag="wexp", name=f"we{li}_{cm['gi0']}")
                    nc.scalar.activation(out=wexp[:], in_=araw[:], func=ACTF.Exp)
                    nc.scalar.activation(out=wstack[:, :, 0:H], in_=araw[:], func=ACTF.Exp)
                    # weighted messages from raw gathered features
                    if glo:
                        nc.vector.tensor_tensor(
                            out=wstack[:, 0:glo, H:].rearrange("p t (h c) -> p t h c", h=H),
                            in0=g_lo[:, 0:glo, :].rearrange("p t (h c) -> p t h c", h=H),
                            in1=wexp[:, 0:glo, :].rearrange("p t h -> p t h ()").broadcast_to([128, glo, H, C]),
                            op=ALU.mult)
                    if ghi:
                        nc.vector.tensor_tensor(
                            out=wstack[:, glo:T, H:].rearrange("p t (h c) -> p t h c", h=H),
                            in0=g_hi[:, 0:ghi, :].rearrange("p t (h c) -> p t h c", h=H),
                            in1=wexp[:, glo:T, :].rearrange("p t h -> p t h ()").broadcast_to([128, ghi, H, C]),
                            op=ALU.mult)

                    # scatter per dst block: [den | num] accumulated on PE,
                    # normalization batched across the chunk's blocks
                    CB = len(cm['blks'])
                    b0 = cm['blks'][0]
                    acc = psE.tile([128, CB, H + HID], F32, tag="eacc", name=f"acc{li}_{cm['gi0']}")
                    for bi, (b, slots) in enumerate(cm['per_block']):
                        for j, t in enumerate(slots):
                            nc.tensor.matmul(acc[:, bi, :], oh_ch[:, t, :], wstack[:, t, :],
                                             start=(j == 0), stop=(j == len(slots) - 1))
                    den = workp.tile([128, CB, H], F32, tag="den", name=f"den{li}_{cm['gi0']}")
                    nc.vector.tensor_scalar(out=den[:], in0=acc[:, :, 0:H], scalar1=1e-16, scalar2=None, op0=ALU.add)
                    recip = workp.tile([128, CB, H], F32, tag="recip", name=f"rc{li}_{cm['gi0']}")
                    nc.vector.reciprocal(out=recip[:], in_=den[:])
                    xn = workp.tile([128, CB, HID], F32, tag="xn", name=f"xn{li}_{cm['gi0']}")
                    nc.vector.tensor_tensor(
                        out=xn[:].rearrange("p b (h c) -> p b h c", h=H),
                        in0=acc[:, :, H:].rearrange("p b (h c) -> p b h c", h=H),
                        in1=recip[:].rearrange("p b h -> p b h ()").broadcast_to([128, CB, H, C]),
                        op=ALU.mult)
                    nc.vector.tensor_tensor(
                        out=xn[:], in0=xn[:],
                        in1=b_rep.rearrange("p f -> p () f").broadcast_to([128, CB, HID]),
                        op=ALU.add)
                    if relu:
                        nc.scalar.activation(out=xslice[:, b0:b0 + CB, :], in_=xn[:], func=ACTF.Relu)
                    else:
                        nc.scalar.copy(out=xslice[:, b0:b0 + CB, :], in_=xn[:])

                if li < NLAYERS - 1:
                    Wn = c['Wn_all'][:, li * HID:(li + 1) * HID]
                    xpown = slicep.tile([128, BPC, HID], BF, tag="xpown", name=f"xpown{li + 1}")
                    for b in range(BPC):
                        tp = psT.tile([128, 128], BF, tag="tp", name=f"ntp{li}_{b}")
                        nc.tensor.transpose(tp[:], xslice[:, b, :], c['ident_bf'][:])
                        xT = workp.tile([128, 128], BF, tag="xT", name=f"nxT{li}_{b}")
                        nc.scalar.copy(out=xT[:], in_=tp[:])
                        xpp = psMM.tile([128, HID], F32, tag="mm", name=f"nxpp{li}_{b}")
                        nc.tensor.matmul(xpp[:], xT[:], Wn, start=True, stop=True)
                        nc.scalar.copy(out=xpown[:, b, :], in_=xpp[:])

            # ---------------- MLP head + pool ----------------
            gp = psG.tile([NG, 1], F32, tag="gp", name="gp")
            for b in range(BPC):
                tp = psT.tile([128, 128], BF, tag="tp", name=f"mtp{b}")
                nc.tensor.transpose(tp[:], xslice[:, b, :], c['ident_bf'][:])
                xT = workp.tile([128, 128], BF, tag="xT", name=f"mxT{b}")
                nc.scalar.copy(out=xT[:], in_=tp[:])
                h1 = []
                for jh in range(2):
                    hp = psMM.tile([128, 128], F32, tag="mm", name=f"mh1p{b}_{jh}")
                    nc.tensor.matmul(hp[:], c['C1w'][:, jh * 128:(jh + 1) * 128], xT[:],
                                     start=True, stop=True)
                    hs = workp.tile([128, 128], BF, tag=f"h1_{jh}", name=f"mh1s{b}_{jh}")
                    nc.vector.tensor_scalar(out=hs[:], in0=hp[:],
                                            scalar1=c['c1b_col'][:, jh:jh + 1],
                                            scalar2=0.0, op0=ALU.add, op1=ALU.max)
                    h1.append(hs)
                h2 = []
                for jh in range(2):
                    hp = psMM.tile([128, 128], F32, tag="mm", name=f"mh2p{b}_{jh}")
                    for kc in range(2):
                        nc.tensor.matmul(hp[:], c['C2w'][:, kc * FC + jh * 128:kc * FC + (jh + 1) * 128],
                                         h1[kc][:], start=(kc == 0), stop=(kc == 1))
                    hs = workp.tile([128, 128], BF, tag=f"h2_{jh}", name=f"mh2s{b}_{jh}")
                    nc.vector.tensor_scalar(out=hs[:], in0=hp[:],
                                            scalar1=c['c2b_col'][:, jh:jh + 1],
                                            scalar2=0.0, op0=ALU.add, op1=ALU.max)
                    h2.append(hs)
                nvp = psMM.tile([128, 1], F32, tag="mm", name=f"mnvp{b}")
                for kc in range(2):
                    nc.tensor.matmul(nvp[:], h2[kc][:], c['C3w'][:, kc:kc + 1],
                                     start=(kc == 0), stop=(kc == 1))
                nv = workp.tile([128, 1], BF, tag="nv", name=f"mnv{b}")
                nc.vector.tensor_scalar(out=nv[:], in0=nvp[:], scalar1=host['C3b'],
                                        scalar2=0.0, op0=ALU.add, op1=ALU.max)
                nc.tensor.matmul(gp[:], c['pool_mat'][:, b * NG:(b + 1) * NG], nv[:],
                                 start=(b == 0), stop=(b == BPC - 1))
            pt = workp.tile([NG, 1], F32, tag="pt", name="pt")
            nc.scalar.copy(out=pt[:], in_=gp[:])
            nc.sync.dma_start(t_part.ap(), pt[:])

    nc.compile()
    return nc


def kernel(**inputs):
    from concourse.bass_utils import run_bass_kernel_spmd
    import hashlib
    inputs = {k: np.asarray(v) for k, v in inputs.items()}
    host = _build_host(inputs)
    key = hashlib.sha1(np.ascontiguousarray(inputs['edge_index']).tobytes()).hexdigest()
    if key not in _cache:
        _cache[key] = _build_program(host)
    prog = _cache[key]

    in_maps = []
    for k in range(NCORES):
        perm = host['perms'][k]
        in_maps.append(dict(
            idx_lo=host['idx_lo'][k], idx_hi=host['idx_hi'][k],
            idx_dst=host['idx_dst'][k],
            dstcol=host['dstcol'][k], latcol=host['latcol'][k],
            a_s_all=host['a_s_all'], a_d_all=host['a_d_all'],
            b_all=host['b_all'], we_all=host['we_all'], Wn_all=host['Wn_all'],
            req_w=np.ascontiguousarray(host['req_w_full'][:, perm]),
            us_own=np.ascontiguousarray(host['us_w_full'][:, k * BPC:(k + 1) * BPC]),
            mask_ge15=np.ascontiguousarray(host['mask_ge15'][:, perm]),
            mask_lt15=np.ascontiguousarray(host['mask_lt15'][:, perm]),
            onehot4T=host['onehot4T'][k], T0=host['T0'],
            w16_rep=host['w16_rep'], w17_rep=host['w17_rep'],
            C1w=host['C1w'], C2w=host['C2w'], C3w=host['C3w'],
            c1b_col=host['c1b_col'], c2b_col=host['c2b_col'],
            pool_mat=host['pool_mat'][k],
            iota_row_bf=host['iota_row_bf'], ident_bf=host['ident_bf'],
            ones_col=host['ones_col'],
        ))
    def _launch():
        t0 = time.monotonic()
        res = run_bass_kernel_spmd(prog, in_maps, core_ids=list(range(NCORES)))
        wall = (time.monotonic() - t0) * 1e9
        t = res.exec_time_ns if res.exec_time_ns else wall
        p = sum(np.asarray(res.results[k]['partials'], np.float64) for k in range(NCORES))
        return p, t

    # The axon terminal occasionally returns corrupted results right after a
    # device reset; run twice (second launch is cheap in-process) and verify.
    times = []
    p1, t = _launch()
    times.append(t)
    p2, t = _launch()
    times.append(t)
    if not np.allclose(p1, p2, rtol=1e-3, atol=1e-6):
        p3, t = _launch()
        times.append(t)
        if np.allclose(p2, p3, rtol=1e-3, atol=1e-6):
            p1 = p2
        elif np.allclose(p1, p3, rtol=1e-3, atol=1e-6):
            pass
        else:
            p1 = p3
    partials = p1
    out = (partials[:, 0] / np.maximum(host['cnt'], 1.0)).astype(np.float32)[:, None]
    kernel._last_times = times
    return out


# revision 6
# speedup vs baseline: 5.1112x; 1.4634x over previous
"""CriticSwapGNN Trainium2 fused kernel: feat + 4 GAT layers + MLP head + pool
in ONE launch across 8 cores, with on-device AllGather of xp between layers.

Sharding: dst-range ownership, 8 cores x 49 blocks of 128 nodes. Edges sorted
by dst block, split lo/hi by src half (int16 gather indices), tiled 128/tile,
grouped in chunks of CHUNK_BLKS dst blocks. Per chunk: dma_gather of src rows
from the AllGathered table plus dst rows from the core-local slice (s_src and
s_dst both computed per-edge on DVE), segment softmax without max-subtraction
(logits are bounded), one fused [den|num] scatter matmul per tile into PSUM.
Table and matmul operands in bf16 (rel err ~4.5e-3 vs 2e-2 gate); accumulations
in fp32 PSUM. The launch is run twice and cross-checked (the axon terminal
occasionally returns corrupted results right after a device reset).
"""
import sys
import time
import numpy as np
import ml_dtypes

if '/opt/trn_rl_repo' not in sys.path:
    sys.path.insert(0, '/opt/trn_rl_repo')

N = 50000; E = 800000; F = 16; HID = 128; H = 4; C = 32; FC = 256; NL = 15; NG = 8
NCORES = 8
BLK = 128
BPC = 49                      # blocks per core
NBLK = NCORES * BPC           # 392
NPAD = NBLK * BLK             # 50176
HALF = 4 * BPC * BLK          # 25088
CHUNK_BLKS = 2
NLAYERS = 4

_cache = {}
BF16 = ml_dtypes.bfloat16


def _chunk_layout():
    chunks = []
    b = 0
    while b < BPC:
        chunks.append(list(range(b, min(b + CHUNK_BLKS, BPC))))
        b += CHUNK_BLKS
    return chunks


def _build_host(inputs):
    src = np.asarray(inputs['edge_index'][0], np.int64)
    dst = np.asarray(inputs['edge_index'][1], np.int64)
    lat = np.asarray(inputs['latency'], np.float32)

    order = np.argsort(dst, kind='stable')
    es, ed, el = src[order], dst[order], lat[order]
    blk_of = ed // BLK
    blk_starts = np.searchsorted(blk_of, np.arange(NBLK + 1))

    # per (core, block): lo/hi edge lists
    per = {}
    nlo = np.zeros((NCORES, BPC), np.int64)
    nhi = np.zeros((NCORES, BPC), np.int64)
    for k in range(NCORES):
        for b in range(BPC):
            g = k * BPC + b
            s_, e_ = blk_starts[g], blk_starts[g + 1]
            bs, bd, bl = es[s_:e_], ed[s_:e_] - g * BLK, el[s_:e_]
            lo = bs < HALF
            per[(k, b)] = (bs[lo], bd[lo], bl[lo], bs[~lo] - HALF, bd[~lo], bl[~lo])
            nlo[k, b] = len(bs[lo])
            nhi[k, b] = len(bs) - nlo[k, b]
    TLO = -(-nlo.max(axis=0) // 128)      # uniform tiles per block (lo half)
    THI = -(-nhi.max(axis=0) // 128)

    chunks = _chunk_layout()
    # global tile axis: chunk-major; within chunk: lo tiles (block-major), hi tiles
    tile_axis = []
    for blks in chunks:
        for b in blks:
            for t in range(int(TLO[b])):
                tile_axis.append(('lo', b, t))
        for b in blks:
            for t in range(int(THI[b])):
                tile_axis.append(('hi', b, t))
    ntiles = len(tile_axis)
    gi_of = {v: i for i, v in enumerate(tile_axis)}
    lo_order = [v for v in tile_axis if v[0] == 'lo']   # gather order, chunk-major
    hi_order = [v for v in tile_axis if v[0] == 'hi']
    lo_pos = {v: i for i, v in enumerate(lo_order)}
    hi_pos = {v: i for i, v in enumerate(hi_order)}
    n_lo_tiles, n_hi_tiles = len(lo_order), len(hi_order)

    idx_lo = np.zeros((NCORES, 128, n_lo_tiles * 8), np.int16)
    idx_hi = np.zeros((NCORES, 128, n_hi_tiles * 8), np.int16)
    idx_dst = np.zeros((NCORES, 128, ntiles * 8), np.int16)
    dstcol = np.full((NCORES, 128, ntiles), float(BLK), np.float32)
    latcol = np.zeros((NCORES, 128, ntiles), np.float32)

    def wrap16(a):     # [128] int -> [128, 8] int16 wrapped+replicated
        return np.tile(a.astype(np.int16).reshape(-1, 16).T, (8, 1))

    for k in range(NCORES):
        for b in range(BPC):
            slo, dlo, llo, shi, dhi, lhi = per[(k, b)]
            for half, s_, d_, l_, T_, pos, idx_arr in (
                    ('lo', slo, dlo, llo, TLO, lo_pos, idx_lo),
                    ('hi', shi, dhi, lhi, THI, hi_pos, idx_hi)):
                nt = int(T_[b])
                if nt == 0:
                    continue
                cap = nt * 128
                sp = np.zeros(cap, np.int64)
                dp = np.full(cap, BLK, np.int64)
                dl = np.zeros(cap, np.int64)        # dst local node (pad -> 0)
                lp = np.zeros(cap, np.float32)
                sp[:len(s_)] = s_
                dp[:len(d_)] = d_
                dl[:len(d_)] = b * BLK + d_
                lp[:len(l_)] = l_
                for t in range(nt):
                    p = pos[(half, b, t)]
                    idx_arr[k][:, p * 8:(p + 1) * 8] = wrap16(sp[t * 128:(t + 1) * 128])
                    gi = gi_of[(half, b, t)]
                    idx_dst[k][:, gi * 8:(gi + 1) * 8] = wrap16(dl[t * 128:(t + 1) * 128])
                    dstcol[k][:, gi] = dp[t * 128:(t + 1) * 128]
                    latcol[k][:, gi] = lp[t * 128:(t + 1) * 128]

    # ---- features ----
    type_ids = np.asarray(inputs['type_ids'], np.int64)
    node = np.arange(NPAD)
    valid = node < N
    k_ = node // (BPC * BLK)
    b_ = (node % (BPC * BLK)) // BLK
    p_ = node % BLK
    onehot4T = np.zeros((NCORES, 4, BPC * BLK), np.float32)
    tid = np.full(NPAD, -1, np.int64)
    tid[:N] = type_ids
    for t in range(4):
        m = tid == t
        onehot4T[k_[m], t, b_[m] * BLK + p_[m]] = 1.0

    def wrapnode(x):   # [N] -> [128, NBLK]
        o = np.zeros(NPAD, np.float32)
        o[:N] = x
        return o.reshape(-1, 128).T.copy()

    req_w_full = wrapnode(np.asarray(inputs['requests'], np.float32))
    us_w_full = wrapnode(np.asarray(inputs['update_step'], np.float32))
    idx_node = np.arange(NPAD).reshape(-1, 128).T
    mask_ge15 = ((idx_node >= NL) & (idx_node < N)).astype(np.float32)
    mask_lt15 = (idx_node < NL).astype(np.float32)

    perms = []
    for k in range(NCORES):
        own = np.arange(k * BPC, (k + 1) * BPC)
        rest = np.array([c for c in range(NBLK) if not (k * BPC <= c < (k + 1) * BPC)])
        perms.append(np.concatenate([own, rest]))

    def rep_row(v):    # [HID] -> [128, HID]
        return np.tile(np.asarray(v, np.float32).reshape(1, -1), (128, 1))

    def we_fold(We, a_e):
        We = np.asarray(We, np.float32).reshape(1, HID)
        a_e = np.asarray(a_e, np.float32)
        return np.array([(We[0, h * C:(h + 1) * C] * a_e[h]).sum() for h in range(H)],
                        np.float32)

    W0 = np.asarray(inputs['W0'], np.float32)
    T0 = (np.asarray(inputs['emb'], np.float32) @ W0[:F]).astype(np.float32)

    Ls = []
    Ls.append(dict(a_s=np.asarray(inputs['as0'], np.float32).reshape(HID),
                   a_d=np.asarray(inputs['ad0'], np.float32).reshape(HID),
                   we=we_fold(inputs['We0'], inputs['ae0']),
                   b=np.asarray(inputs['b0'], np.float32),
                   Wn=np.asarray(inputs['Wh'][0], np.float32)))
    Ls.append(dict(a_s=np.asarray(inputs['ash'][0], np.float32).reshape(HID),
                   a_d=np.asarray(inputs['adh'][0], np.float32).reshape(HID),
                   we=we_fold(np.asarray(inputs['Weh'][0]).reshape(1, -1), inputs['aeh'][0]),
                   b=np.asarray(inputs['bh'][0], np.float32),
                   Wn=np.asarray(inputs['Wh'][1], np.float32)))
    Ls.append(dict(a_s=np.asarray(inputs['ash'][1], np.float32).reshape(HID),
                   a_d=np.asarray(inputs['adh'][1], np.float32).reshape(HID),
                   we=we_fold(np.asarray(inputs['Weh'][1]).reshape(1, -1), inputs['aeh'][1]),
                   b=np.asarray(inputs['bh'][1], np.float32),
                   Wn=np.asarray(inputs['Wf'], np.float32)))
    Ls.append(dict(a_s=np.asarray(inputs['asf'], np.float32).reshape(HID),
                   a_d=np.asarray(inputs['adf'], np.float32).reshape(HID),
                   we=we_fold(inputs['Wef'], inputs['aef']),
                   b=np.asarray(inputs['bf'], np.float32),
                   Wn=None))

    a_s_all = np.concatenate([rep_row(L['a_s']) for L in Ls], axis=1).astype(BF16)
    a_d_all = np.concatenate([rep_row(L['a_d']) for L in Ls], axis=1).astype(BF16)
    b_all = np.concatenate([rep_row(L['b']) for L in Ls], axis=1).astype(np.float32)
    we_all = np.concatenate([np.tile(L['we'].reshape(1, H), (128, 1)) for L in Ls],
                            axis=1).astype(np.float32)
    Wn_all = np.concatenate([Ls[i]['Wn'] for i in range(3)], axis=1).astype(BF16)

    batch = np.asarray(inputs['batch'], np.int64)
    cnt = np.zeros(NG, np.float64)
    np.add.at(cnt, batch, 1.0)
    pool_mat = np.zeros((NCORES, 128, BPC * NG), np.float32)
    bv = batch[node[valid]]
    pool_mat[k_[valid], p_[valid], b_[valid] * NG + bv] = 1.0

    C2w = np.asarray(inputs['C2w'], np.float32)
    host = dict(
        TLO=TLO, THI=THI, chunks=chunks, ntiles=ntiles,
        n_lo_tiles=n_lo_tiles, n_hi_tiles=n_hi_tiles,
        idx_lo=idx_lo, idx_hi=idx_hi, idx_dst=idx_dst, dstcol=dstcol.astype(BF16),
        latcol=latcol,
        onehot4T=onehot4T.astype(BF16), req_w_full=req_w_full, us_w_full=us_w_full,
        mask_ge15=mask_ge15, mask_lt15=mask_lt15, perms=perms,
        T0=T0.astype(BF16),
        w16_rep=rep_row(np.tile(W0[F], 1)), w17_rep=rep_row(W0[F + 1]),
        a_s_all=a_s_all, a_d_all=a_d_all, b_all=b_all, we_all=we_all,
        Wn_all=Wn_all, cnt=cnt, pool_mat=pool_mat.astype(BF16),
        C1w=np.asarray(inputs['C1w'], np.float32).astype(BF16),
        C2w=np.concatenate([C2w[0:128], C2w[128:256]], axis=1).astype(BF16),
        C3w=np.ascontiguousarray(np.asarray(inputs['C3w'], np.float32).reshape(2, 128).T).astype(BF16),
        c1b_col=np.ascontiguousarray(np.asarray(inputs['C1b'], np.float32).reshape(2, 128).T),
        c2b_col=np.ascontiguousarray(np.asarray(inputs['C2b'], np.float32).reshape(2, 128).T),
        C3b=float(np.asarray(inputs['C3b'], np.float32)[0]),
        iota_row_bf=np.tile(np.arange(128, dtype=np.float32)[None, :], (128, 1)).astype(BF16),
        ident_bf=np.eye(128, dtype=np.float32).astype(BF16),
        ones_col=np.ones((128, 1), np.float32),
    )
    return host


def _build_program(host):
    import concourse.bacc as bacc
    import concourse.mybir as mybir
    import concourse.tile as tile
    from concourse import library_config
    F32 = mybir.dt.float32
    BF = mybir.dt.bfloat16
    I16 = mybir.dt.int16
    ALU = mybir.AluOpType
    AX = mybir.AxisListType
    ACTF = mybir.ActivationFunctionType

    TLO, THI, chunks = host['TLO'], host['THI'], host['chunks']
    ntiles = host['ntiles']
    nlo, nhi = host['n_lo_tiles'], host['n_hi_tiles']

    nc = bacc.Bacc("TRN2", target_bir_lowering=False, debug=False, num_devices=NCORES)

    ins = {
        'idx_lo': ([128, nlo * 8], I16), 'idx_hi': ([128, nhi * 8], I16),
        'idx_dst': ([128, ntiles * 8], I16),
        'dstcol': ([128, ntiles], BF), 'latcol': ([128, ntiles], F32),
        'a_s_all': ([128, NLAYERS * HID], BF), 'a_d_all': ([128, NLAYERS * HID], BF),
        'b_all': ([128, NLAYERS * HID], F32), 'we_all': ([128, NLAYERS * H], F32),
        'Wn_all': ([128, 3 * HID], BF),
        'req_w': ([128, NBLK], F32), 'us_own': ([128, BPC], F32),
        'mask_ge15': ([128, NBLK], F32), 'mask_lt15': ([128, NBLK], F32),
        'onehot4T': ([4, BPC * BLK], BF), 'T0': ([4, HID], BF),
        'w16_rep': ([128, HID], F32), 'w17_rep': ([128, HID], F32),
        'C1w': ([HID, FC], BF), 'C2w': ([128, 2 * FC], BF), 'C3w': ([128, 2], BF),
        'c1b_col': ([128, 2], F32), 'c2b_col': ([128, 2], F32),
        'pool_mat': ([128, BPC * NG], BF),
        'iota_row_bf': ([128, 128], BF), 'ident_bf': ([128, 128], BF),
        'ones_col': ([128, 1], F32),
    }
    tin = {}
    for name, (shape, dt) in ins.items():
        tin[name] = nc.dram_tensor(name, list(shape), dt, kind="ExternalInput")
    t_part = nc.dram_tensor('partials', [NG, 1], F32, kind="ExternalOutput")

    # static chunk metadata: per chunk -> (lo_tile_start, glo, hi_tile_start, ghi,
    #   per-block slot lists, global tile index base)
    chunk_meta = []
    lo_base = hi_base = gi_base = 0
    for blks in chunks:
        glo = int(sum(TLO[b] for b in blks))
        ghi = int(sum(THI[b] for b in blks))
        T = glo + ghi
        # chunk slot s in [0,T): lo slots first (block-major), then hi
        per_block = []     # (b, [slots], [is_lo flags])
        s = 0
        slots_lo = {}
        for b in blks:
            slots_lo[b] = list(range(s, s + int(TLO[b])))
            s += int(TLO[b])
        slots_hi = {}
        for b in blks:
            slots_hi[b] = list(range(s, s + int(THI[b])))
            s += int(THI[b])
        for b in blks:
            per_block.append((b, slots_lo[b] + slots_hi[b]))
        chunk_meta.append(dict(blks=blks, glo=glo, ghi=ghi, T=T,
                               lo0=lo_base, hi0=hi_base, gi0=gi_base,
                               per_block=per_block))
        lo_base += glo
        hi_base += ghi
        gi_base += T

    with tile.TileContext(nc) as tc:
        with (
            tc.tile_pool(name="const", bufs=1) as constp,
            tc.tile_pool(name="gbuf", bufs=2) as gbufp,
            tc.tile_pool(name="ohp", bufs=2) as ohp,
            tc.tile_pool(name="wsp", bufs=2) as wsp,
            tc.tile_pool(name="work", bufs=2) as workp,
            tc.tile_pool(name="slice", bufs=2) as slicep,
            tc.tile_pool(name="psT", bufs=2, space="PSUM") as psT,
            tc.tile_pool(name="psE", bufs=3, space="PSUM") as psE,
            tc.tile_pool(name="psG", bufs=1, space="PSUM") as psG,
            tc.tile_pool(name="psMM", bufs=2, space="PSUM") as psMM,
            tc.tile_pool(name="dram", bufs=2, space="DRAM") as dramp,
        ):
            nc.gpsimd.load_library(library_config.mlp)
            c = {}
            for name in ins:
                shape, dt = ins[name]
                t = constp.tile(list(shape), dt, tag=name, name=name)
                nc.sync.dma_start(t[:], tin[name].ap())
                c[name] = t

            # ---------------- feat phase: xp1 own slice ----------------
            n = float(N - NL)
            d = workp.tile([128, NBLK], F32, tag="fd", name="fd")
            nc.vector.tensor_tensor(out=d[:], in0=c['req_w'][:], in1=c['mask_ge15'][:], op=ALU.mult)
            col = workp.tile([128, 1], F32, tag="fcol", name="fcol")
            nc.vector.tensor_reduce(out=col[:], in_=d[:], op=ALU.add, axis=AX.X)
            tot = psMM.tile([1, 1], F32, tag="mm", name="ftot")
            nc.tensor.matmul(tot[:], col[:], c['ones_col'][:], start=True, stop=True)
            mean = workp.tile([1, 1], F32, tag="fmean", name="fmean")
            nc.vector.tensor_scalar(out=mean[:], in0=tot[:], scalar1=1.0 / n, scalar2=None, op0=ALU.mult)
            mean_col = workp.tile([128, 1], F32, tag="fmc", name="fmc")
            nc.gpsimd.partition_broadcast(mean_col[:], mean[:])
            nc.vector.tensor_scalar(out=d[:], in0=c['req_w'][:], scalar1=mean_col[:, 0:1], scalar2=None, op0=ALU.subtract)
            nc.vector.tensor_tensor(out=d[:], in0=d[:], in1=c['mask_ge15'][:], op=ALU.mult)
            d2 = workp.tile([128, NBLK], F32, tag="fd2", name="fd2")
            nc.vector.tensor_tensor(out=d2[:], in0=d[:], in1=d[:], op=ALU.mult)
            nc.vector.tensor_reduce(out=col[:], in_=d2[:], op=ALU.add, axis=AX.X)
            tot2 = psMM.tile([1, 1], F32, tag="mm", name="ftot2")
            nc.tensor.matmul(tot2[:], col[:], c['ones_col'][:], start=True, stop=True)
            var = workp.tile([1, 1], F32, tag="fvar", name="fvar")
            nc.vector.tensor_scalar(out=var[:], in0=tot2[:], scalar1=1.0 / (n - 1.0), scalar2=None, op0=ALU.mult)
            std = workp.tile([1, 1], F32, tag="fstd", name="fstd")
            nc.scalar.activation(out=std[:], in_=var[:], func=ACTF.Sqrt)
            nc.vector.tensor_scalar(out=std[:], in0=std[:], scalar1=1e-6, scalar2=None, op0=ALU.add)
            rinv = workp.tile([1, 1], F32, tag="frinv", name="frinv")
            nc.vector.reciprocal(out=rinv[:], in_=std[:])
            rinv_col = workp.tile([128, 1], F32, tag="frc", name="frc")
            nc.gpsimd.partition_broadcast(rinv_col[:], rinv[:])
            rf = workp.tile([128, NBLK], F32, tag="frf", name="frf")
            nc.vector.tensor_scalar(out=rf[:], in0=d[:], scalar1=rinv_col[:, 0:1], scalar2=None, op0=ALU.mult)
            raw15 = workp.tile([128, NBLK], F32, tag="fr15", name="fr15")
            nc.vector.tensor_tensor(out=raw15[:], in0=c['req_w'][:], in1=c['mask_lt15'][:], op=ALU.mult)
            nc.vector.tensor_tensor(out=rf[:], in0=rf[:], in1=raw15[:], op=ALU.add)

            xpown = slicep.tile([128, BPC, HID], BF, tag="xpown", name="xpown0")
            for b in range(BPC):
                mm = psMM.tile([128, HID], F32, tag="mm", name="fmm")
                nc.tensor.matmul(mm[:], c['onehot4T'][:, b * 128:(b + 1) * 128], c['T0'][:],
                                 start=True, stop=True)
                x0 = workp.tile([128, HID], F32, tag="fx0", name="fx0")
                t1 = workp.tile([128, HID], F32, tag="ft1", name="ft1")
                nc.vector.tensor_scalar(out=t1[:], in0=c['w16_rep'][:], scalar1=rf[:, b:b + 1], scalar2=None, op0=ALU.mult)
                nc.vector.tensor_tensor(out=x0[:], in0=mm[:], in1=t1[:], op=ALU.add)
                nc.vector.tensor_scalar(out=t1[:], in0=c['w17_rep'][:], scalar1=c['us_own'][:, b:b + 1], scalar2=None, op0=ALU.mult)
                nc.vector.tensor_tensor(out=x0[:], in0=x0[:], in1=t1[:], op=ALU.add)
                nc.scalar.copy(out=xpown[:, b, :], in_=x0[:])

            # ---------------- 4 GAT layers ----------------
            for li in range(NLAYERS):
                a_s = c['a_s_all'][:, li * HID:(li + 1) * HID]
                a_d = c['a_d_all'][:, li * HID:(li + 1) * HID]
                b_rep = c['b_all'][:, li * HID:(li + 1) * HID]
                we_rep = c['we_all'][:, li * H:(li + 1) * H]
                relu = li < NLAYERS - 1

                # ship own xp slice, AllGather full table
                xsl = dramp.tile([BPC * BLK, HID], BF, tag="xsl", name=f"xsl{li}")
                nc.sync.dma_start(xsl[:].rearrange("(b p) j -> p b j", p=128), xpown[:])
                tab = dramp.tile([NPAD, HID], BF, tag="tab", addr_space="Shared", name=f"tab{li}")
                nc.gpsimd.collective_compute(
                    "AllGather", ALU.bypass,
                    replica_groups=[list(range(NCORES))],
                    ins=[xsl[:]], outs=[tab[:]])

                xslice = slicep.tile([128, BPC, HID], BF, tag="xslice", name=f"xslice{li}", bufs=1)

                for cm in chunk_meta:
                    glo, ghi, T = cm['glo'], cm['ghi'], cm['T']
                    g_lo = gbufp.tile([128, max(glo, 1), HID], BF, tag="g_lo", name=f"glo{li}_{cm['gi0']}")
                    g_hi = gbufp.tile([128, max(ghi, 1), HID], BF, tag="g_hi", name=f"ghi{li}_{cm['gi0']}")
                    if glo:
                        nc.gpsimd.dma_gather(
                            g_lo[:, 0:glo, :], tab[0:HALF, :],
                            c['idx_lo'][:, cm['lo0'] * 8:(cm['lo0'] + glo) * 8],
                            glo * 128, glo * 128, HID, single_packet=False)
                    if ghi:
                        nc.gpsimd.dma_gather(
                            g_hi[:, 0:ghi, :], tab[HALF:NPAD, :],
                            c['idx_hi'][:, cm['hi0'] * 8:(cm['hi0'] + ghi) * 8],
                            ghi * 128, ghi * 128, HID, single_packet=False)
                    # dst rows from the core-local slice (no AG dependency)
                    g_dst = gbufp.tile([128, T, HID], BF, tag="g_dst", name=f"gd{li}_{cm['gi0']}")
                    nc.gpsimd.dma_gather(
                        g_dst[:], xsl[:, :],
                        c['idx_dst'][:, cm['gi0'] * 8:(cm['gi0'] + T) * 8],
                        T * 128, T * 128, HID, single_packet=False)

                    # s_src for all chunk slots (lo block-major, then hi)
                    s_src = workp.tile([128, T, H], F32, tag="s_src", name=f"ss{li}_{cm['gi0']}")
                    xa = workp.tile([128, T, HID], BF, tag="xa", name=f"xa{li}_{cm['gi0']}", bufs=1)
                    if glo:
                        nc.vector.tensor_tensor(
                            out=xa[:, 0:glo, :], in0=g_lo[:, 0:glo, :],
                            in1=a_s.rearrange("p f -> p () f").broadcast_to([128, glo, HID]),
                            op=ALU.mult)
                    if ghi:
                        nc.vector.tensor_tensor(
                            out=xa[:, glo:T, :], in0=g_hi[:, 0:ghi, :],
                            in1=a_s.rearrange("p f -> p () f").broadcast_to([128, ghi, HID]),
                            op=ALU.mult)
                    nc.vector.tensor_reduce(
                        out=s_src[:], in_=xa[:].rearrange("p t (h c) -> p t h c", h=H),
                        op=ALU.add, axis=AX.X)

                    # one-hot dst matrices for the chunk
                    oh_ch = ohp.tile([128, T, 128], BF, tag="oh_ch", name=f"oh{li}_{cm['gi0']}")
                    nc.vector.tensor_tensor(
                        out=oh_ch[:],
                        in0=c['iota_row_bf'][:].rearrange("p f -> p () f").broadcast_to([128, T, 128]),
                        in1=c['dstcol'][:, cm['gi0']:cm['gi0'] + T].rearrange("p t -> p t ()").broadcast_to([128, T, 128]),
                        op=ALU.is_equal)

                    # s_dst per edge from gathered dst rows
                    s_dst = workp.tile([128, T, H], F32, tag="s_dst", name=f"sd{li}_{cm['gi0']}")
                    xad = workp.tile([128, T, HID], BF, tag="xa", name=f"xad{li}_{cm['gi0']}", bufs=1)
                    nc.vector.tensor_tensor(
                        out=xad[:], in0=g_dst[:],
                        in1=a_d.rearrange("p f -> p () f").broadcast_to([128, T, HID]),
                        op=ALU.mult)
                    nc.vector.tensor_reduce(
                        out=s_dst[:], in_=xad[:].rearrange("p t (h c) -> p t h c", h=H),
                        op=ALU.add, axis=AX.X)

                    # alpha: s_src + s_dst + we*lat; leaky_relu; exp
                    latw = workp.tile([128, T, H], F32, tag="latw", name=f"lw{li}_{cm['gi0']}")
                    nc.vector.tensor_tensor(
                        out=latw[:],
                        in0=c['latcol'][:, cm['gi0']:cm['gi0'] + T].rearrange("p t -> p t ()").broadcast_to([128, T, H]),
                        in1=we_rep.rearrange("p h -> p () h").broadcast_to([128, T, H]),
                        op=ALU.mult)
                    araw = workp.tile([128, T, H], F32, tag="araw", name=f"ar{li}_{cm['gi0']}")
                    nc.vector.tensor_tensor(out=araw[:], in0=s_src[:], in1=s_dst[:], op=ALU.add)
                    nc.vector.tensor_tensor(out=araw[:], in0=araw[:], in1=latw[:], op=ALU.add)
                    lr = workp.tile([128, T, H], F32, tag="lr", name=f"lr{li}_{cm['gi0']}")
                    nc.vector.tensor_scalar(out=lr[:], in0=araw[:], scalar1=0.2, scalar2=None, op0=ALU.mult)
                    nc.vector.tensor_tensor(out=araw[:], in0=araw[:], in1=lr[:], op=ALU.max)

                    wstack = wsp.tile([128, T, H + HID], BF, tag="wstack", name=f"wst{li}_{cm['gi0']}")
                    wexp = workp.tile([128, T, H], BF, tag="wexp", name=f"we{li}_{cm['gi0']}")
                    nc.scalar.activation(out=wexp[:], in_=araw[:], func=ACTF.Exp)
                    nc.scalar.activation(out=wstack[:, :, 0:H], in_=araw[:], func=ACTF.Exp)
                    # weighted messages from raw gathered features
                    if glo:
                        nc.vector.tensor_tensor(
                            out=wstack[:, 0:glo, H:].rearrange("p t (h c) -> p t h c", h=H),
                            in0=g_lo[:, 0:glo, :].rearrange("p t (h c) -> p t h c", h=H),
                            in1=wexp[:, 0:glo, :].rearrange("p t h -> p t h ()").broadcast_to([128, glo, H, C]),
                            op=ALU.mult)
                    if ghi:
                        nc.vector.tensor_tensor(
                            out=wstack[:, glo:T, H:].rearrange("p t (h c) -> p t h c", h=H),
                            in0=g_hi[:, 0:ghi, :].rearrange("p t (h c) -> p t h c", h=H),
                            in1=wexp[:, glo:T, :].rearrange("p t h -> p t h ()").broadcast_to([128, ghi, H, C]),
                            op=ALU.mult)

                    # scatter per dst block: [den | num] accumulated on PE,
                    # normalization batched across the chunk's blocks
                    CB = len(cm['blks'])
                    b0 = cm['blks'][0]
                    acc = psE.tile([128, CB, H + HID], F32, tag="eacc", name=f"acc{li}_{cm['gi0']}")
                    for bi, (b, slots) in enumerate(cm['per_block']):
                        for j, t in enumerate(slots):
                            nc.tensor.matmul(acc[:, bi, :], oh_ch[:, t, :], wstack[:, t, :],
                                             start=(j == 0), stop=(j == len(slots) - 1))
                    den = workp.tile([128, CB, H], F32, tag="den", name=f"den{li}_{cm['gi0']}")
                    nc.vector.tensor_scalar(out=den[:], in0=acc[:, :, 0:H], scalar1=1e-16, scalar2=None, op0=ALU.add)
                    recip = workp.tile([128, CB, H], F32, tag="recip", name=f"rc{li}_{cm['gi0']}")
                    nc.vector.reciprocal(out=recip[:], in_=den[:])
                    xn = workp.tile([128, CB, HID], F32, tag="xn", name=f"xn{li}_{cm['gi0']}")
                    nc.vector.tensor_tensor(
                        out=xn[:].rearrange("p b (h c) -> p b h c", h=H),
                        in0=acc[:, :, H:].rearrange("p b (h c) -> p b h c", h=H),
                        in1=recip[:].rearrange("p b h -> p b h ()").broadcast_to([128, CB, H, C]),
                        op=ALU.mult)
                    nc.vector.tensor_tensor(
                        out=xn[:], in0=xn[:],
                        in1=b_rep.rearrange("p f -> p () f").broadcast_to([128, CB, HID]),
                        op=ALU.add)
                    if relu:
                        nc.scalar.activation(out=xslice[:, b0:b0 + CB, :], in_=xn[:], func=ACTF.Relu)
                    else:
                        nc.scalar.copy(out=xslice[:, b0:b0 + CB, :], in_=xn[:])

                if li < NLAYERS - 1:
                    Wn = c['Wn_all'][:, li * HID:(li + 1) * HID]
                    xpown = slicep.tile([128, BPC, HID], BF, tag="xpown", name=f"xpown{li + 1}")
                    for b in range(BPC):
                        tp = psT.tile([128, 128], BF, tag="tp", name=f"ntp{li}_{b}")
                        nc.tensor.transpose(tp[:], xslice[:, b, :], c['ident_bf'][:])
                        xT = workp.tile([128, 128], BF, tag="xT", name=f"nxT{li}_{b}")
                        nc.scalar.copy(out=xT[:], in_=tp[:])
                        xpp = psMM.tile([128, HID], F32, tag="mm", name=f"nxpp{li}_{b}")
                        nc.tensor.matmul(xpp[:], xT[:], Wn, start=True, stop=True)
                        nc.scalar.copy(out=xpown[:, b, :], in_=xpp[:])

            # ---------------- MLP head + pool ----------------
            gp = psG.tile([NG, 1], F32, tag="gp", name="gp")
            for b in range(BPC):
                tp = psT.tile([128, 128], BF, tag="tp", name=f"mtp{b}")
                nc.tensor.transpose(tp[:], xslice[:, b, :], c['ident_bf'][:])
                xT = workp.tile([128, 128], BF, tag="xT", name=f"mxT{b}")
                nc.scalar.copy(out=xT[:], in_=tp[:])
                h1 = []
                for jh in range(2):
                    hp = psMM.tile([128, 128], F32, tag="mm", name=f"mh1p{b}_{jh}")
                    nc.tensor.matmul(hp[:], c['C1w'][:, jh * 128:(jh + 1) * 128], xT[:],
                                     start=True, stop=True)
                    hs = workp.tile([128, 128], BF, tag=f"h1_{jh}", name=f"mh1s{b}_{jh}")
                    nc.vector.tensor_scalar(out=hs[:], in0=hp[:],
                                            scalar1=c['c1b_col'][:, jh:jh + 1],
                                            scalar2=0.0, op0=ALU.add, op1=ALU.max)
                    h1.append(hs)
                h2 = []
                for jh in range(2):
                    hp = psMM.tile([128, 128], F32, tag="mm", name=f"mh2p{b}_{jh}")
                    for kc in range(2):
                        nc.tensor.matmul(hp[:], c['C2w'][:, kc * FC + jh * 128:kc * FC + (jh + 1) * 128],
                                         h1[kc][:], start=(kc == 0), stop=(kc == 1))
                    hs = workp.tile([128, 128], BF, tag=f"h2_{jh}", name=f"mh2s{b}_{jh}")
                    nc.vector.tensor_scalar(out=hs[:], in0=hp[:],
                                            scalar1=c['c2b_col'][:, jh:jh + 1],
                                            scalar2=0.0, op0=ALU.add, op1=ALU.max)
                    h2.append(hs)
                nvp = psMM.tile([128, 1], F32, tag="mm", name=f"mnvp{b}")
                for kc in range(2):
                    nc.tensor.matmul(nvp[:], h2[kc][:], c['C3w'][:, kc:kc + 1],
                                     start=(kc == 0), stop=(kc == 1))
                nv = workp.tile([128, 1], BF, tag="nv", name=f"mnv{b}")
                nc.vector.tensor_scalar(out=nv[:], in0=nvp[:], scalar1=host['C3b'],
                                        scalar2=0.0, op0=ALU.add, op1=ALU.max)
                nc.tensor.matmul(gp[:], c['pool_mat'][:, b * NG:(b + 1) * NG], nv[:],
                                 start=(b == 0), stop=(b == BPC - 1))
            pt = workp.tile([NG, 1], F32, tag="pt", name="pt")
            nc.scalar.copy(out=pt[:], in_=gp[:])
            nc.sync.dma_start(t_part.ap(), pt[:])

    nc.compile()
    return nc


def kernel(**inputs):
    from concourse.bass_utils import run_bass_kernel_spmd
    import hashlib
    inputs = {k: np.asarray(v) for k, v in inputs.items()}
    host = _build_host(inputs)
    key = hashlib.sha1(np.ascontiguousarray(inputs['edge_index']).tobytes()).hexdigest()
    if key not in _cache:
        _cache[key] = _build_program(host)
    prog = _cache[key]

    in_maps = []
    for k in range(NCORES):
        perm = host['perms'][k]
        in_maps.append(dict(
            idx_lo=host['idx_lo'][k], idx_hi=host['idx_hi'][k],
            idx_dst=host['idx_dst'][k],
            dstcol=host['dstcol'][k], latcol=host['latcol'][k],
            a_s_all=host['a_s_all'], a_d_all=host['a_d_all'],
            b_all=host['b_all'], we_all=host['we_all'], Wn_all=host['Wn_all'],
            req_w=np.ascontiguousarray(host['req_w_full'][:, perm]),
            us_own=np.ascontiguousarray(host['us_w_full'][:, k * BPC:(k + 1) * BPC]),
            mask_ge15=np.ascontiguousarray(host['mask_ge15'][:, perm]),
            mask_lt15=np.ascontiguousarray(host['mask_lt15'][:, perm]),
            onehot4T=host['onehot4T'][k], T0=host['T0'],
            w16_rep=host['w16_rep'], w17_rep=host['w17_rep'],
            C1w=host['C1w'], C2w=host['C2w'], C3w=host['C3w'],
            c1b_col=host['c1b_col'], c2b_col=host['c2b_col'],
            pool_mat=host['pool_mat'][k],
            iota_row_bf=host['iota_row_bf'], ident_bf=host['ident_bf'],
            ones_col=host['ones_col'],
        ))
    def _launch():
        t0 = time.monotonic()
        try:
            res = run_bass_kernel_spmd(prog, in_maps, core_ids=list(range(NCORES)))
        except Exception:
            time.sleep(5.0)   # transient device wedge: one retry
            res = run_bass_kernel_spmd(prog, in_maps, core_ids=list(range(NCORES)))
        wall = (time.monotonic() - t0) * 1e9
        t = res.exec_time_ns if res.exec_time_ns else wall
        p = sum(np.asarray(res.results[k]['partials'], np.float64) for k in range(NCORES))
        return p, t

    # The axon terminal occasionally returns corrupted results right after a
    # device reset; run twice (second launch is cheap in-process) and verify.
    times = []
    p1, t = _launch()
    times.append(t)
    p2, t = _launch()
    times.append(t)
    if not np.allclose(p1, p2, rtol=1e-3, atol=1e-6):
        p3, t = _launch()
        times.append(t)
        if np.allclose(p2, p3, rtol=1e-3, atol=1e-6):
            p1 = p2
        elif np.allclose(p1, p3, rtol=1e-3, atol=1e-6):
            pass
        else:
            p1 = p3
    partials = p1
    out = (partials[:, 0] / np.maximum(host['cnt'], 1.0)).astype(np.float32)[:, None]
    kernel._last_times = times
    return out


# revision 7
# speedup vs baseline: 5.2045x; 1.0182x over previous
"""CriticSwapGNN Trainium2 fused kernel: feat + 4 GAT layers + MLP head + pool
in ONE launch across 8 cores, with on-device AllGather of xp between layers.

Sharding: dst-range ownership, 8 cores x 49 blocks of 128 nodes. Edges sorted
by dst block, split lo/hi by src half (int16 gather indices), tiled 128/tile,
grouped in chunks of CHUNK_BLKS dst blocks. Per chunk: dma_gather of src rows
from the AllGathered table plus dst rows from the core-local slice (s_src and
s_dst both computed per-edge on DVE), segment softmax without max-subtraction
(logits are bounded), one fused [den|num] scatter matmul per tile into PSUM.
Table and matmul operands in bf16 (rel err ~4.5e-3 vs 2e-2 gate); accumulations
in fp32 PSUM. The launch is run twice and cross-checked (the axon terminal
occasionally returns corrupted results right after a device reset).
"""
import sys
import time
import numpy as np
import ml_dtypes

if '/opt/trn_rl_repo' not in sys.path:
    sys.path.insert(0, '/opt/trn_rl_repo')

N = 50000; E = 800000; F = 16; HID = 128; H = 4; C = 32; FC = 256; NL = 15; NG = 8
NCORES = 8
BLK = 128
BPC = 49                      # blocks per core
NBLK = NCORES * BPC           # 392
NPAD = NBLK * BLK             # 50176
HALF = 4 * BPC * BLK          # 25088
CHUNK_BLKS = 2
NLAYERS = 4

_cache = {}
BF16 = ml_dtypes.bfloat16


def _chunk_layout():
    chunks = []
    b = 0
    while b < BPC:
        chunks.append(list(range(b, min(b + CHUNK_BLKS, BPC))))
        b += CHUNK_BLKS
    return chunks


def _build_host(inputs):
    src = np.asarray(inputs['edge_index'][0], np.int64)
    dst = np.asarray(inputs['edge_index'][1], np.int64)
    lat = np.asarray(inputs['latency'], np.float32)

    order = np.argsort(dst, kind='stable')
    es, ed, el = src[order], dst[order], lat[order]
    blk_of = ed // BLK
    blk_starts = np.searchsorted(blk_of, np.arange(NBLK + 1))

    # per (core, block): lo/hi edge lists
    per = {}
    nlo = np.zeros((NCORES, BPC), np.int64)
    nhi = np.zeros((NCORES, BPC), np.int64)
    for k in range(NCORES):
        for b in range(BPC):
            g = k * BPC + b
            s_, e_ = blk_starts[g], blk_starts[g + 1]
            bs, bd, bl = es[s_:e_], ed[s_:e_] - g * BLK, el[s_:e_]
            lo = bs < HALF
            per[(k, b)] = (bs[lo], bd[lo], bl[lo], bs[~lo] - HALF, bd[~lo], bl[~lo])
            nlo[k, b] = len(bs[lo])
            nhi[k, b] = len(bs) - nlo[k, b]
    TLO = -(-nlo.max(axis=0) // 128)      # uniform tiles per block (lo half)
    THI = -(-nhi.max(axis=0) // 128)

    chunks = _chunk_layout()
    # global tile axis: chunk-major; within chunk: lo tiles (block-major), hi tiles
    tile_axis = []
    for blks in chunks:
        for b in blks:
            for t in range(int(TLO[b])):
                tile_axis.append(('lo', b, t))
        for b in blks:
            for t in range(int(THI[b])):
                tile_axis.append(('hi', b, t))
    ntiles = len(tile_axis)
    gi_of = {v: i for i, v in enumerate(tile_axis)}
    lo_order = [v for v in tile_axis if v[0] == 'lo']   # gather order, chunk-major
    hi_order = [v for v in tile_axis if v[0] == 'hi']
    lo_pos = {v: i for i, v in enumerate(lo_order)}
    hi_pos = {v: i for i, v in enumerate(hi_order)}
    n_lo_tiles, n_hi_tiles = len(lo_order), len(hi_order)

    idx_lo = np.zeros((NCORES, 128, n_lo_tiles * 8), np.int16)
    idx_hi = np.zeros((NCORES, 128, n_hi_tiles * 8), np.int16)
    idx_dst = np.zeros((NCORES, 128, ntiles * 8), np.int16)
    dstcol = np.full((NCORES, 128, ntiles), float(BLK), np.float32)
    latcol = np.zeros((NCORES, 128, ntiles), np.float32)

    def wrap16(a):     # [128] int -> [128, 8] int16 wrapped+replicated
        return np.tile(a.astype(np.int16).reshape(-1, 16).T, (8, 1))

    for k in range(NCORES):
        for b in range(BPC):
            slo, dlo, llo, shi, dhi, lhi = per[(k, b)]
            for half, s_, d_, l_, T_, pos, idx_arr in (
                    ('lo', slo, dlo, llo, TLO, lo_pos, idx_lo),
                    ('hi', shi, dhi, lhi, THI, hi_pos, idx_hi)):
                nt = int(T_[b])
                if nt == 0:
                    continue
                cap = nt * 128
                sp = np.zeros(cap, np.int64)
                dp = np.full(cap, BLK, np.int64)
                dl = np.zeros(cap, np.int64)        # dst local node (pad -> 0)
                lp = np.zeros(cap, np.float32)
                sp[:len(s_)] = s_
                dp[:len(d_)] = d_
                dl[:len(d_)] = b * BLK + d_
                lp[:len(l_)] = l_
                for t in range(nt):
                    p = pos[(half, b, t)]
                    idx_arr[k][:, p * 8:(p + 1) * 8] = wrap16(sp[t * 128:(t + 1) * 128])
                    gi = gi_of[(half, b, t)]
                    idx_dst[k][:, gi * 8:(gi + 1) * 8] = wrap16(dl[t * 128:(t + 1) * 128])
                    dstcol[k][:, gi] = dp[t * 128:(t + 1) * 128]
                    latcol[k][:, gi] = lp[t * 128:(t + 1) * 128]

    # ---- features ----
    type_ids = np.asarray(inputs['type_ids'], np.int64)
    node = np.arange(NPAD)
    valid = node < N
    k_ = node // (BPC * BLK)
    b_ = (node % (BPC * BLK)) // BLK
    p_ = node % BLK
    onehot4T = np.zeros((NCORES, 4, BPC * BLK), np.float32)
    tid = np.full(NPAD, -1, np.int64)
    tid[:N] = type_ids
    for t in range(4):
        m = tid == t
        onehot4T[k_[m], t, b_[m] * BLK + p_[m]] = 1.0

    def wrapnode(x):   # [N] -> [128, NBLK]
        o = np.zeros(NPAD, np.float32)
        o[:N] = x
        return o.reshape(-1, 128).T.copy()

    req_w_full = wrapnode(np.asarray(inputs['requests'], np.float32))
    us_w_full = wrapnode(np.asarray(inputs['update_step'], np.float32))
    idx_node = np.arange(NPAD).reshape(-1, 128).T
    mask_ge15 = ((idx_node >= NL) & (idx_node < N)).astype(np.float32)
    mask_lt15 = (idx_node < NL).astype(np.float32)

    perms = []
    for k in range(NCORES):
        own = np.arange(k * BPC, (k + 1) * BPC)
        rest = np.array([c for c in range(NBLK) if not (k * BPC <= c < (k + 1) * BPC)])
        perms.append(np.concatenate([own, rest]))

    def rep_row(v):    # [HID] -> [128, HID]
        return np.tile(np.asarray(v, np.float32).reshape(1, -1), (128, 1))

    def we_fold(We, a_e):
        We = np.asarray(We, np.float32).reshape(1, HID)
        a_e = np.asarray(a_e, np.float32)
        return np.array([(We[0, h * C:(h + 1) * C] * a_e[h]).sum() for h in range(H)],
                        np.float32)

    W0 = np.asarray(inputs['W0'], np.float32)
    T0 = (np.asarray(inputs['emb'], np.float32) @ W0[:F]).astype(np.float32)

    Ls = []
    Ls.append(dict(a_s=np.asarray(inputs['as0'], np.float32).reshape(HID),
                   a_d=np.asarray(inputs['ad0'], np.float32).reshape(HID),
                   we=we_fold(inputs['We0'], inputs['ae0']),
                   b=np.asarray(inputs['b0'], np.float32),
                   Wn=np.asarray(inputs['Wh'][0], np.float32)))
    Ls.append(dict(a_s=np.asarray(inputs['ash'][0], np.float32).reshape(HID),
                   a_d=np.asarray(inputs['adh'][0], np.float32).reshape(HID),
                   we=we_fold(np.asarray(inputs['Weh'][0]).reshape(1, -1), inputs['aeh'][0]),
                   b=np.asarray(inputs['bh'][0], np.float32),
                   Wn=np.asarray(inputs['Wh'][1], np.float32)))
    Ls.append(dict(a_s=np.asarray(inputs['ash'][1], np.float32).reshape(HID),
                   a_d=np.asarray(inputs['adh'][1], np.float32).reshape(HID),
                   we=we_fold(np.asarray(inputs['Weh'][1]).reshape(1, -1), inputs['aeh'][1]),
                   b=np.asarray(inputs['bh'][1], np.float32),
                   Wn=np.asarray(inputs['Wf'], np.float32)))
    Ls.append(dict(a_s=np.asarray(inputs['asf'], np.float32).reshape(HID),
                   a_d=np.asarray(inputs['adf'], np.float32).reshape(HID),
                   we=we_fold(inputs['Wef'], inputs['aef']),
                   b=np.asarray(inputs['bf'], np.float32),
                   Wn=None))

    a_s_all = np.concatenate([rep_row(L['a_s']) for L in Ls], axis=1).astype(BF16)
    a_d_all = np.concatenate([rep_row(L['a_d']) for L in Ls], axis=1).astype(BF16)
    b_all = np.concatenate([rep_row(L['b']) for L in Ls], axis=1).astype(np.float32)
    we_all = np.concatenate([np.tile(L['we'].reshape(1, H), (128, 1)) for L in Ls],
                            axis=1).astype(np.float32)
    Wn_all = np.concatenate([Ls[i]['Wn'] for i in range(3)], axis=1).astype(BF16)

    batch = np.asarray(inputs['batch'], np.int64)
    cnt = np.zeros(NG, np.float64)
    np.add.at(cnt, batch, 1.0)
    pool_mat = np.zeros((NCORES, 128, BPC * NG), np.float32)
    bv = batch[node[valid]]
    pool_mat[k_[valid], p_[valid], b_[valid] * NG + bv] = 1.0

    C2w = np.asarray(inputs['C2w'], np.float32)
    host = dict(
        TLO=TLO, THI=THI, chunks=chunks, ntiles=ntiles,
        n_lo_tiles=n_lo_tiles, n_hi_tiles=n_hi_tiles,
        idx_lo=idx_lo, idx_hi=idx_hi, idx_dst=idx_dst, dstcol=dstcol.astype(BF16),
        latcol=latcol,
        onehot4T=onehot4T.astype(BF16), req_w_full=req_w_full, us_w_full=us_w_full,
        mask_ge15=mask_ge15, mask_lt15=mask_lt15, perms=perms,
        T0=T0.astype(BF16),
        w16_rep=rep_row(np.tile(W0[F], 1)), w17_rep=rep_row(W0[F + 1]),
        a_s_all=a_s_all, a_d_all=a_d_all, b_all=b_all, we_all=we_all,
        Wn_all=Wn_all, cnt=cnt, pool_mat=pool_mat.astype(BF16),
        C1w=np.asarray(inputs['C1w'], np.float32).astype(BF16),
        C2w=np.concatenate([C2w[0:128], C2w[128:256]], axis=1).astype(BF16),
        C3w=np.ascontiguousarray(np.asarray(inputs['C3w'], np.float32).reshape(2, 128).T).astype(BF16),
        c1b_col=np.ascontiguousarray(np.asarray(inputs['C1b'], np.float32).reshape(2, 128).T),
        c2b_col=np.ascontiguousarray(np.asarray(inputs['C2b'], np.float32).reshape(2, 128).T),
        C3b=float(np.asarray(inputs['C3b'], np.float32)[0]),
        iota_row_bf=np.tile(np.arange(128, dtype=np.float32)[None, :], (128, 1)).astype(BF16),
        ident_bf=np.eye(128, dtype=np.float32).astype(BF16),
        ones_col=np.ones((128, 1), np.float32),
    )
    return host


def _build_program(host):
    import concourse.bacc as bacc
    import concourse.mybir as mybir
    import concourse.tile as tile
    from concourse import library_config
    F32 = mybir.dt.float32
    BF = mybir.dt.bfloat16
    I16 = mybir.dt.int16
    ALU = mybir.AluOpType
    AX = mybir.AxisListType
    ACTF = mybir.ActivationFunctionType

    TLO, THI, chunks = host['TLO'], host['THI'], host['chunks']
    ntiles = host['ntiles']
    nlo, nhi = host['n_lo_tiles'], host['n_hi_tiles']

    nc = bacc.Bacc("TRN2", target_bir_lowering=False, debug=False, num_devices=NCORES)

    ins = {
        'idx_lo': ([128, nlo * 8], I16), 'idx_hi': ([128, nhi * 8], I16),
        'idx_dst': ([128, ntiles * 8], I16),
        'dstcol': ([128, ntiles], BF), 'latcol': ([128, ntiles], F32),
        'a_s_all': ([128, NLAYERS * HID], BF), 'a_d_all': ([128, NLAYERS * HID], BF),
        'b_all': ([128, NLAYERS * HID], F32), 'we_all': ([128, NLAYERS * H], F32),
        'Wn_all': ([128, 3 * HID], BF),
        'req_w': ([128, NBLK], F32), 'us_own': ([128, BPC], F32),
        'mask_ge15': ([128, NBLK], F32), 'mask_lt15': ([128, NBLK], F32),
        'onehot4T': ([4, BPC * BLK], BF), 'T0': ([4, HID], BF),
        'w16_rep': ([128, HID], F32), 'w17_rep': ([128, HID], F32),
        'C1w': ([HID, FC], BF), 'C2w': ([128, 2 * FC], BF), 'C3w': ([128, 2], BF),
        'c1b_col': ([128, 2], F32), 'c2b_col': ([128, 2], F32),
        'pool_mat': ([128, BPC * NG], BF),
        'iota_row_bf': ([128, 128], BF), 'ident_bf': ([128, 128], BF),
        'ones_col': ([128, 1], F32),
    }
    tin = {}
    for name, (shape, dt) in ins.items():
        tin[name] = nc.dram_tensor(name, list(shape), dt, kind="ExternalInput")
    t_part = nc.dram_tensor('partials', [NG, 1], F32, kind="ExternalOutput")

    # static chunk metadata: per chunk -> (lo_tile_start, glo, hi_tile_start, ghi,
    #   per-block slot lists, global tile index base)
    chunk_meta = []
    lo_base = hi_base = gi_base = 0
    for blks in chunks:
        glo = int(sum(TLO[b] for b in blks))
        ghi = int(sum(THI[b] for b in blks))
        T = glo + ghi
        # chunk slot s in [0,T): lo slots first (block-major), then hi
        per_block = []     # (b, [slots], [is_lo flags])
        s = 0
        slots_lo = {}
        for b in blks:
            slots_lo[b] = list(range(s, s + int(TLO[b])))
            s += int(TLO[b])
        slots_hi = {}
        for b in blks:
            slots_hi[b] = list(range(s, s + int(THI[b])))
            s += int(THI[b])
        for b in blks:
            per_block.append((b, slots_lo[b] + slots_hi[b]))
        chunk_meta.append(dict(blks=blks, glo=glo, ghi=ghi, T=T,
                               lo0=lo_base, hi0=hi_base, gi0=gi_base,
                               per_block=per_block))
        lo_base += glo
        hi_base += ghi
        gi_base += T

    with tile.TileContext(nc) as tc:
        with (
            tc.tile_pool(name="const", bufs=1) as constp,
            tc.tile_pool(name="gbuf", bufs=2) as gbufp,
            tc.tile_pool(name="ohp", bufs=2) as ohp,
            tc.tile_pool(name="wsp", bufs=2) as wsp,
            tc.tile_pool(name="work", bufs=2) as workp,
            tc.tile_pool(name="slice", bufs=2) as slicep,
            tc.tile_pool(name="psT", bufs=2, space="PSUM") as psT,
            tc.tile_pool(name="psE", bufs=3, space="PSUM") as psE,
            tc.tile_pool(name="psG", bufs=1, space="PSUM") as psG,
            tc.tile_pool(name="psMM", bufs=2, space="PSUM") as psMM,
            tc.tile_pool(name="dram", bufs=2, space="DRAM") as dramp,
        ):
            nc.gpsimd.load_library(library_config.mlp)
            c = {}
            for name in ins:
                shape, dt = ins[name]
                t = constp.tile(list(shape), dt, tag=name, name=name)
                nc.sync.dma_start(t[:], tin[name].ap())
                c[name] = t

            # ---------------- feat phase: xp1 own slice ----------------
            n = float(N - NL)
            d = workp.tile([128, NBLK], F32, tag="fd", name="fd")
            nc.vector.tensor_tensor(out=d[:], in0=c['req_w'][:], in1=c['mask_ge15'][:], op=ALU.mult)
            col = workp.tile([128, 1], F32, tag="fcol", name="fcol")
            nc.vector.tensor_reduce(out=col[:], in_=d[:], op=ALU.add, axis=AX.X)
            tot = psMM.tile([1, 1], F32, tag="mm", name="ftot")
            nc.tensor.matmul(tot[:], col[:], c['ones_col'][:], start=True, stop=True)
            mean = workp.tile([1, 1], F32, tag="fmean", name="fmean")
            nc.vector.tensor_scalar(out=mean[:], in0=tot[:], scalar1=1.0 / n, scalar2=None, op0=ALU.mult)
            mean_col = workp.tile([128, 1], F32, tag="fmc", name="fmc")
            nc.gpsimd.partition_broadcast(mean_col[:], mean[:])
            nc.vector.tensor_scalar(out=d[:], in0=c['req_w'][:], scalar1=mean_col[:, 0:1], scalar2=None, op0=ALU.subtract)
            nc.vector.tensor_tensor(out=d[:], in0=d[:], in1=c['mask_ge15'][:], op=ALU.mult)
            d2 = workp.tile([128, NBLK], F32, tag="fd2", name="fd2")
            nc.vector.tensor_tensor(out=d2[:], in0=d[:], in1=d[:], op=ALU.mult)
            nc.vector.tensor_reduce(out=col[:], in_=d2[:], op=ALU.add, axis=AX.X)
            tot2 = psMM.tile([1, 1], F32, tag="mm", name="ftot2")
            nc.tensor.matmul(tot2[:], col[:], c['ones_col'][:], start=True, stop=True)
            var = workp.tile([1, 1], F32, tag="fvar", name="fvar")
            nc.vector.tensor_scalar(out=var[:], in0=tot2[:], scalar1=1.0 / (n - 1.0), scalar2=None, op0=ALU.mult)
            std = workp.tile([1, 1], F32, tag="fstd", name="fstd")
            nc.scalar.activation(out=std[:], in_=var[:], func=ACTF.Sqrt)
            nc.vector.tensor_scalar(out=std[:], in0=std[:], scalar1=1e-6, scalar2=None, op0=ALU.add)
            rinv = workp.tile([1, 1], F32, tag="frinv", name="frinv")
            nc.vector.reciprocal(out=rinv[:], in_=std[:])
            rinv_col = workp.tile([128, 1], F32, tag="frc", name="frc")
            nc.gpsimd.partition_broadcast(rinv_col[:], rinv[:])
            rf = workp.tile([128, NBLK], F32, tag="frf", name="frf")
            nc.vector.tensor_scalar(out=rf[:], in0=d[:], scalar1=rinv_col[:, 0:1], scalar2=None, op0=ALU.mult)
            raw15 = workp.tile([128, NBLK], F32, tag="fr15", name="fr15")
            nc.vector.tensor_tensor(out=raw15[:], in0=c['req_w'][:], in1=c['mask_lt15'][:], op=ALU.mult)
            nc.vector.tensor_tensor(out=rf[:], in0=rf[:], in1=raw15[:], op=ALU.add)

            xpown = slicep.tile([128, BPC, HID], BF, tag="xpown", name="xpown0")
            for b in range(BPC):
                mm = psMM.tile([128, HID], F32, tag="mm", name="fmm")
                nc.tensor.matmul(mm[:], c['onehot4T'][:, b * 128:(b + 1) * 128], c['T0'][:],
                                 start=True, stop=True)
                x0 = workp.tile([128, HID], F32, tag="fx0", name="fx0")
                t1 = workp.tile([128, HID], F32, tag="ft1", name="ft1")
                nc.vector.tensor_scalar(out=t1[:], in0=c['w16_rep'][:], scalar1=rf[:, b:b + 1], scalar2=None, op0=ALU.mult)
                nc.vector.tensor_tensor(out=x0[:], in0=mm[:], in1=t1[:], op=ALU.add)
                nc.vector.tensor_scalar(out=t1[:], in0=c['w17_rep'][:], scalar1=c['us_own'][:, b:b + 1], scalar2=None, op0=ALU.mult)
                nc.vector.tensor_tensor(out=x0[:], in0=x0[:], in1=t1[:], op=ALU.add)
                nc.scalar.copy(out=xpown[:, b, :], in_=x0[:])

            # ---------------- 4 GAT layers ----------------
            for li in range(NLAYERS):
                a_s = c['a_s_all'][:, li * HID:(li + 1) * HID]
                a_d = c['a_d_all'][:, li * HID:(li + 1) * HID]
                b_rep = c['b_all'][:, li * HID:(li + 1) * HID]
                we_rep = c['we_all'][:, li * H:(li + 1) * H]
                relu = li < NLAYERS - 1

                # ship own xp slice, AllGather full table
                xsl = dramp.tile([BPC * BLK, HID], BF, tag="xsl", name=f"xsl{li}")
                nc.sync.dma_start(xsl[:].rearrange("(b p) j -> p b j", p=128), xpown[:])
                tab = dramp.tile([NPAD, HID], BF, tag="tab", addr_space="Shared", name=f"tab{li}")
                nc.gpsimd.collective_compute(
                    "AllGather", ALU.bypass,
                    replica_groups=[list(range(NCORES))],
                    ins=[xsl[:]], outs=[tab[:]])

                xslice = slicep.tile([128, BPC, HID], BF, tag="xslice", name=f"xslice{li}", bufs=1)

                for cm in chunk_meta:
                    glo, ghi, T = cm['glo'], cm['ghi'], cm['T']
                    g_lo = gbufp.tile([128, max(glo, 1), HID], BF, tag="g_lo", name=f"glo{li}_{cm['gi0']}")
                    g_hi = gbufp.tile([128, max(ghi, 1), HID], BF, tag="g_hi", name=f"ghi{li}_{cm['gi0']}")
                    if glo:
                        nc.gpsimd.dma_gather(
                            g_lo[:, 0:glo, :], tab[0:HALF, :],
                            c['idx_lo'][:, cm['lo0'] * 8:(cm['lo0'] + glo) * 8],
                            glo * 128, glo * 128, HID, single_packet=False)
                    if ghi:
                        nc.gpsimd.dma_gather(
                            g_hi[:, 0:ghi, :], tab[HALF:NPAD, :],
                            c['idx_hi'][:, cm['hi0'] * 8:(cm['hi0'] + ghi) * 8],
                            ghi * 128, ghi * 128, HID, single_packet=False)
                    # dst rows from the core-local slice (no AG dependency)
                    g_dst = gbufp.tile([128, T, HID], BF, tag="g_dst", name=f"gd{li}_{cm['gi0']}")
                    nc.gpsimd.dma_gather(
                        g_dst[:], xsl[:, :],
                        c['idx_dst'][:, cm['gi0'] * 8:(cm['gi0'] + T) * 8],
                        T * 128, T * 128, HID, single_packet=False)

                    # s_src for all chunk slots (lo block-major, then hi)
                    s_src = workp.tile([128, T, H], F32, tag="s_src", name=f"ss{li}_{cm['gi0']}")
                    xa = workp.tile([128, T, HID], BF, tag="xa", name=f"xa{li}_{cm['gi0']}", bufs=1)
                    if glo:
                        nc.vector.tensor_tensor(
                            out=xa[:, 0:glo, :], in0=g_lo[:, 0:glo, :],
                            in1=a_s.rearrange("p f -> p () f").broadcast_to([128, glo, HID]),
                            op=ALU.mult)
                    if ghi:
                        nc.vector.tensor_tensor(
                            out=xa[:, glo:T, :], in0=g_hi[:, 0:ghi, :],
                            in1=a_s.rearrange("p f -> p () f").broadcast_to([128, ghi, HID]),
                            op=ALU.mult)
                    nc.vector.tensor_reduce(
                        out=s_src[:], in_=xa[:].rearrange("p t (h c) -> p t h c", h=H),
                        op=ALU.add, axis=AX.X)

                    # one-hot dst matrices for the chunk
                    oh_ch = ohp.tile([128, T, 128], BF, tag="oh_ch", name=f"oh{li}_{cm['gi0']}")
                    nc.vector.tensor_tensor(
                        out=oh_ch[:],
                        in0=c['iota_row_bf'][:].rearrange("p f -> p () f").broadcast_to([128, T, 128]),
                        in1=c['dstcol'][:, cm['gi0']:cm['gi0'] + T].rearrange("p t -> p t ()").broadcast_to([128, T, 128]),
                        op=ALU.is_equal)

                    # s_dst per edge from gathered dst rows
                    s_dst = workp.tile([128, T, H], F32, tag="s_dst", name=f"sd{li}_{cm['gi0']}")
                    xad = workp.tile([128, T, HID], BF, tag="xa", name=f"xad{li}_{cm['gi0']}", bufs=1)
                    nc.vector.tensor_tensor(
                        out=xad[:], in0=g_dst[:],
                        in1=a_d.rearrange("p f -> p () f").broadcast_to([128, T, HID]),
                        op=ALU.mult)
                    nc.vector.tensor_reduce(
                        out=s_dst[:], in_=xad[:].rearrange("p t (h c) -> p t h c", h=H),
                        op=ALU.add, axis=AX.X)

                    # alpha: s_src + s_dst + we*lat; leaky_relu; exp
                    latw = workp.tile([128, T, H], F32, tag="latw", name=f"lw{li}_{cm['gi0']}")
                    nc.vector.tensor_tensor(
                        out=latw[:],
                        in0=c['latcol'][:, cm['gi0']:cm['gi0'] + T].rearrange("p t -> p t ()").broadcast_to([128, T, H]),
                        in1=we_rep.rearrange("p h -> p () h").broadcast_to([128, T, H]),
                        op=ALU.mult)
                    araw = workp.tile([128, T, H], F32, tag="araw", name=f"ar{li}_{cm['gi0']}")
                    nc.vector.tensor_tensor(out=araw[:], in0=s_src[:], in1=s_dst[:], op=ALU.add)
                    nc.vector.tensor_tensor(out=araw[:], in0=araw[:], in1=latw[:], op=ALU.add)
                    lr = workp.tile([128, T, H], F32, tag="lr", name=f"lr{li}_{cm['gi0']}")
                    nc.vector.tensor_scalar(out=lr[:], in0=araw[:], scalar1=0.2, scalar2=None, op0=ALU.mult)
                    nc.vector.tensor_tensor(out=araw[:], in0=araw[:], in1=lr[:], op=ALU.max)

                    wstack = wsp.tile([128, T, H + HID], BF, tag="wstack", name=f"wst{li}_{cm['gi0']}")
                    wexp = workp.tile([128, T, H], BF, tag="wexp", name=f"we{li}_{cm['gi0']}")
                    nc.scalar.activation(out=wexp[:], in_=araw[:], func=ACTF.Exp)
                    nc.scalar.activation(out=wstack[:, :, 0:H], in_=araw[:], func=ACTF.Exp)
                    # weighted messages from raw gathered features
                    if glo:
                        nc.vector.tensor_tensor(
                            out=wstack[:, 0:glo, H:].rearrange("p t (h c) -> p t h c", h=H),
                            in0=g_lo[:, 0:glo, :].rearrange("p t (h c) -> p t h c", h=H),
                            in1=wexp[:, 0:glo, :].rearrange("p t h -> p t h ()").broadcast_to([128, glo, H, C]),
                            op=ALU.mult)
                    if ghi:
                        nc.vector.tensor_tensor(
                            out=wstack[:, glo:T, H:].rearrange("p t (h c) -> p t h c", h=H),
                            in0=g_hi[:, 0:ghi, :].rearrange("p t (h c) -> p t h c", h=H),
                            in1=wexp[:, glo:T, :].rearrange("p t h -> p t h ()").broadcast_to([128, ghi, H, C]),
                            op=ALU.mult)

                    # scatter per dst block: [den | num] accumulated on PE,
                    # normalization batched across the chunk's blocks
                    CB = len(cm['blks'])
                    b0 = cm['blks'][0]
                    acc = psE.tile([128, CB, H + HID], F32, tag="eacc", name=f"acc{li}_{cm['gi0']}")
                    for bi, (b, slots) in enumerate(cm['per_block']):
                        for j, t in enumerate(slots):
                            nc.tensor.matmul(acc[:, bi, :], oh_ch[:, t, :], wstack[:, t, :],
                                             start=(j == 0), stop=(j == len(slots) - 1))
                    den = workp.tile([128, CB, H], F32, tag="den", name=f"den{li}_{cm['gi0']}")
                    nc.vector.tensor_scalar(out=den[:], in0=acc[:, :, 0:H], scalar1=1e-16, scalar2=None, op0=ALU.add)
                    recip = workp.tile([128, CB, H], F32, tag="recip", name=f"rc{li}_{cm['gi0']}")
                    nc.vector.reciprocal(out=recip[:], in_=den[:])
                    xn = workp.tile([128, CB, HID], F32, tag="xn", name=f"xn{li}_{cm['gi0']}")
                    nc.vector.tensor_tensor(
                        out=xn[:].rearrange("p b (h c) -> p b h c", h=H),
                        in0=acc[:, :, H:].rearrange("p b (h c) -> p b h c", h=H),
                        in1=recip[:].rearrange("p b h -> p b h ()").broadcast_to([128, CB, H, C]),
                        op=ALU.mult)
                    nc.vector.tensor_tensor(
                        out=xn[:], in0=xn[:],
                        in1=b_rep.rearrange("p f -> p () f").broadcast_to([128, CB, HID]),
                        op=ALU.add)
                    if relu:
                        nc.scalar.activation(out=xslice[:, b0:b0 + CB, :], in_=xn[:], func=ACTF.Relu)
                    else:
                        nc.scalar.copy(out=xslice[:, b0:b0 + CB, :], in_=xn[:])

                if li < NLAYERS - 1:
                    Wn = c['Wn_all'][:, li * HID:(li + 1) * HID]
                    xpown = slicep.tile([128, BPC, HID], BF, tag="xpown", name=f"xpown{li + 1}")
                    for b in range(BPC):
                        tp = psT.tile([128, 128], BF, tag="tp", name=f"ntp{li}_{b}")
                        nc.tensor.transpose(tp[:], xslice[:, b, :], c['ident_bf'][:])
                        xT = workp.tile([128, 128], BF, tag="xT", name=f"nxT{li}_{b}")
                        nc.scalar.copy(out=xT[:], in_=tp[:])
                        xpp = psMM.tile([128, HID], F32, tag="mm", name=f"nxpp{li}_{b}")
                        nc.tensor.matmul(xpp[:], xT[:], Wn, start=True, stop=True)
                        nc.scalar.copy(out=xpown[:, b, :], in_=xpp[:])

            # ---------------- MLP head + pool ----------------
            gp = psG.tile([NG, 1], F32, tag="gp", name="gp")
            for b in range(BPC):
                tp = psT.tile([128, 128], BF, tag="tp", name=f"mtp{b}")
                nc.tensor.transpose(tp[:], xslice[:, b, :], c['ident_bf'][:])
                xT = workp.tile([128, 128], BF, tag="xT", name=f"mxT{b}")
                nc.scalar.copy(out=xT[:], in_=tp[:])
                h1 = []
                for jh in range(2):
                    hp = psMM.tile([128, 128], F32, tag="mm", name=f"mh1p{b}_{jh}")
                    nc.tensor.matmul(hp[:], c['C1w'][:, jh * 128:(jh + 1) * 128], xT[:],
                                     start=True, stop=True)
                    hs = workp.tile([128, 128], BF, tag=f"h1_{jh}", name=f"mh1s{b}_{jh}")
                    nc.vector.tensor_scalar(out=hs[:], in0=hp[:],
                                            scalar1=c['c1b_col'][:, jh:jh + 1],
                                            scalar2=0.0, op0=ALU.add, op1=ALU.max)
                    h1.append(hs)
                h2 = []
                for jh in range(2):
                    hp = psMM.tile([128, 128], F32, tag="mm", name=f"mh2p{b}_{jh}")
                    for kc in range(2):
                        nc.tensor.matmul(hp[:], c['C2w'][:, kc * FC + jh * 128:kc * FC + (jh + 1) * 128],
                                         h1[kc][:], start=(kc == 0), stop=(kc == 1))
                    hs = workp.tile([128, 128], BF, tag=f"h2_{jh}", name=f"mh2s{b}_{jh}")
                    nc.vector.tensor_scalar(out=hs[:], in0=hp[:],
                                            scalar1=c['c2b_col'][:, jh:jh + 1],
                                            scalar2=0.0, op0=ALU.add, op1=ALU.max)
                    h2.append(hs)
                nvp = psMM.tile([128, 1], F32, tag="mm", name=f"mnvp{b}")
                for kc in range(2):
                    nc.tensor.matmul(nvp[:], h2[kc][:], c['C3w'][:, kc:kc + 1],
                                     start=(kc == 0), stop=(kc == 1))
                nv = workp.tile([128, 1], BF, tag="nv", name=f"mnv{b}")
                nc.vector.tensor_scalar(out=nv[:], in0=nvp[:], scalar1=host['C3b'],
                                        scalar2=0.0, op0=ALU.add, op1=ALU.max)
                nc.tensor.matmul(gp[:], c['pool_mat'][:, b * NG:(b + 1) * NG], nv[:],
                                 start=(b == 0), stop=(b == BPC - 1))
            pt = workp.tile([NG, 1], F32, tag="pt", name="pt")
            nc.scalar.copy(out=pt[:], in_=gp[:])
            nc.sync.dma_start(t_part.ap(), pt[:])

    nc.compile()
    return nc


def kernel(**inputs):
    from concourse.bass_utils import run_bass_kernel_spmd
    import hashlib
    # Warm the axon PJRT plugin + device claim up front so one-time runtime
    # init is not conflated with kernel launch time.
    try:
        import jax
        devs = jax.devices()
        jax.device_put(np.zeros(8, np.float32), devs[0]).block_until_ready()
    except Exception:
        pass
    inputs = {k: np.asarray(v) for k, v in inputs.items()}
    host = _build_host(inputs)
    key = hashlib.sha1(np.ascontiguousarray(inputs['edge_index']).tobytes()).hexdigest()
    if key not in _cache:
        _cache[key] = _build_program(host)
    prog = _cache[key]

    in_maps = []
    for k in range(NCORES):
        perm = host['perms'][k]
        in_maps.append(dict(
            idx_lo=host['idx_lo'][k], idx_hi=host['idx_hi'][k],
            idx_dst=host['idx_dst'][k],
            dstcol=host['dstcol'][k], latcol=host['latcol'][k],
            a_s_all=host['a_s_all'], a_d_all=host['a_d_all'],
            b_all=host['b_all'], we_all=host['we_all'], Wn_all=host['Wn_all'],
            req_w=np.ascontiguousarray(host['req_w_full'][:, perm]),
            us_own=np.ascontiguousarray(host['us_w_full'][:, k * BPC:(k + 1) * BPC]),
            mask_ge15=np.ascontiguousarray(host['mask_ge15'][:, perm]),
            mask_lt15=np.ascontiguousarray(host['mask_lt15'][:, perm]),
            onehot4T=host['onehot4T'][k], T0=host['T0'],
            w16_rep=host['w16_rep'], w17_rep=host['w17_rep'],
            C1w=host['C1w'], C2w=host['C2w'], C3w=host['C3w'],
            c1b_col=host['c1b_col'], c2b_col=host['c2b_col'],
            pool_mat=host['pool_mat'][k],
            iota_row_bf=host['iota_row_bf'], ident_bf=host['ident_bf'],
            ones_col=host['ones_col'],
        ))
    def _launch():
        t0 = time.monotonic()
        try:
            res = run_bass_kernel_spmd(prog, in_maps, core_ids=list(range(NCORES)))
        except Exception:
            time.sleep(5.0)   # transient device wedge: one retry
            res = run_bass_kernel_spmd(prog, in_maps, core_ids=list(range(NCORES)))
        wall = (time.monotonic() - t0) * 1e9
        t = res.exec_time_ns if res.exec_time_ns else wall
        p = sum(np.asarray(res.results[k]['partials'], np.float64) for k in range(NCORES))
        return p, t

    # The axon terminal occasionally returns corrupted results right after a
    # device reset; run twice (second launch is cheap in-process) and verify.
    times = []
    p1, t = _launch()
    times.append(t)
    p2, t = _launch()
    times.append(t)
    if not np.allclose(p1, p2, rtol=1e-3, atol=1e-6):
        p3, t = _launch()
        times.append(t)
        if np.allclose(p2, p3, rtol=1e-3, atol=1e-6):
            p1 = p2
        elif np.allclose(p1, p3, rtol=1e-3, atol=1e-6):
            pass
        else:
            p1 = p3
    partials = p1
    out = (partials[:, 0] / np.maximum(host['cnt'], 1.0)).astype(np.float32)[:, None]
    kernel._last_times = times
    return out
